# revision 16
# baseline (speedup 1.0000x reference)
"""Trainium2 Bass kernel for 2D Gaussian Splatting (N=1024, 256x256, 8 cores).

Math: sigma[p,i] is quadratic in pixel coords, so m1 = log(op) - sigma is
a matmul ft[6m,128pix]^T @ g[6m,cols] per 128-pixel block. Consecutive
blocks are merged into block-diagonal groups (contraction 6m <= 126; PE
cost depends only on streamed columns), and coordinates are recentered per
block so every term stays small and fp16 matmul inputs lose no accuracy
(fp32 PE matmul is ~2x slower). alpha = exp(m1) on ACT, bt = alpha*color
on DVE, beta = 1-alpha on ACT (Identity, scale=-1, bias=1); front-to-back
compositing is evaluated back-to-front as the affine scan
C = beta*C + bt along the gaussian axis.

The scan runs ~3.4 cyc/elem on DVE with no 16-bit speedup, so columns are
pre-combined 3 levels (pairs -> quads -> octs) with cheap f16 2x
tensor_tensor ops and the scan covers only L/8 columns. The column layout
is permuted so that pair members at every level are contiguous halves:
position p of a slot goes to section bitreverse3(p % 8).

Culling: 512 blocks of 8x16 pixels; a (gaussian, block) pair is kept iff
the exact minimal sigma over the block rectangle is < 5 (~3.3e-3 image
rel err vs the 2e-2 budget). Non-empty blocks are snake-dealt by width
rank onto the 8 cores (SPMD: identical program, per-core data). Slot
widths are quantized to multiples of 8 and grouped into uniform-width
cohorts; leading sentinel columns (m1=0 -> beta=0) reset the scan state
at every slot start, and one strided copy per cohort gathers the slot
finals.

The For_i timing loop uses staggered_reset (no all-engine barrier per
iteration) and unrolls 8 renders per iteration so tile-pool double
buffering pipelines consecutive renders. Inputs ship compact ([6,*] fp16
coefficient tensors) and are scattered into the block-diagonal SBUF
layout by one-time DMAs. The host reassembles the image from per-core
slot outputs; fully-culled blocks render as zero.
"""

import os
import math
import numpy as np

H = 256
W = 256
N = 1024
NCORES = 8
BR, BC = 8, 16                 # block = 8 rows x 16 cols = 128 pixels
NBY, NBX = H // BR, W // BC
NBLK = NBY * NBX               # 512
SENT_NEG = -80.0
EPS2D = 0.3
ROWS = 7                       # basis rows: x2 xy y2 x y 1 1 (F split hi/lo)
MAXROWS = 126                  # max contraction rows per merged matmul

_cache = {}


# ---------------------------------------------------------------- host math

def _preprocess(means, quats, scales, rgbs, opacities, viewmat, K):
    """Float64 per-gaussian preprocessing. Returns (in back-to-front order):
    G6 [6,N] basis coefficients of log(op)-sigma and colors [N]."""
    md = means.astype(np.float64)
    Rv = viewmat[:3, :3].astype(np.float64)
    t = viewmat[:3, 3].astype(np.float64)
    p_cam = md @ Rv.T + t
    x, y, z = p_cam[:, 0], p_cam[:, 1], p_cam[:, 2]
    fx, fy = float(K[0, 0]), float(K[1, 1])
    cx, cy = float(K[0, 2]), float(K[1, 2])
    inv_z = 1.0 / z
    u = fx * x * inv_z + cx
    v = fy * y * inv_z + cy

    th = quats.astype(np.float64)
    ct, st = np.cos(th), np.sin(th)
    zr, on = np.zeros_like(ct), np.ones_like(ct)
    R3 = np.stack([np.stack([ct, -st, zr], -1),
                   np.stack([st, ct, zr], -1),
                   np.stack([zr, zr, on], -1)], -2)
    M = R3 * scales.astype(np.float64)[:, None, :]
    cov3 = M @ np.swapaxes(M, -1, -2)
    cov_cam = np.einsum('ij,njk,lk->nil', Rv, cov3, Rv)
    j0 = np.stack([fx * inv_z, zr, -fx * x * inv_z * inv_z], -1)
    j1 = np.stack([zr, fy * inv_z, -fy * y * inv_z * inv_z], -1)
    J = np.stack([j0, j1], -2)
    cov2 = np.einsum('nij,njk,nlk->nil', J, cov_cam, J)
    a = cov2[:, 0, 0] + EPS2D
    b = cov2[:, 0, 1]
    c = cov2[:, 1, 1] + EPS2D
    det = a * c - b * b
    ca, cb, cc = c / det, -b / det, a / det

    op = 1.0 / (1.0 + np.exp(-opacities.astype(np.float64)))
    colv = 1.0 / (1.0 + np.exp(-rgbs.astype(np.float64)[:, 0]))

    # reference sorts by fp32 camera z ascending (stable); we composite
    # back-to-front = exact reverse
    order = np.argsort(z.astype(np.float32), kind="stable")
    rev = order[::-1]

    ca2, cc2 = 0.5 * ca, 0.5 * cc
    lop = np.log(op)
    d = -(ca * u + cb * v)
    e = -(cb * u + cc * v)
    f = ca2 * u * u + cb * u * v + cc2 * v * v
    G = np.stack([-ca2, -cb, -cc2, -d, -e, lop - f], 0)[:, rev]  # [6,N] f64
    return G, colv[rev], op[rev], u[rev], v[rev]


def _block_sigma_min(G, u, v):
    """Exact minimal sigma over each block rectangle: 0 if the center is
    inside, else the min over the four edges (1D quadratic, clamped)."""
    ca = -2.0 * G[0]
    cb = -G[1]
    cc = -2.0 * G[2]

    def sigma_at(dx, dy):
        return 0.5 * ca * dx * dx + cb * dx * dy + 0.5 * cc * dy * dy

    smin_all = np.zeros((NBLK, G.shape[1]))
    for by in range(NBY):
        y0, y1 = by * BR + 0.5, by * BR + BR - 0.5
        for bx in range(NBX):
            x0, x1 = bx * BC + 0.5, bx * BC + BC - 0.5
            smin = np.full(G.shape[1], np.inf)
            for xe in (x0, x1):
                dxe = xe - u
                dye = np.clip(-cb * dxe / cc, y0 - v, y1 - v)
                smin = np.minimum(smin, sigma_at(dxe, dye))
            for ye in (y0, y1):
                dye = ye - v
                dxe = np.clip(-cb * dye / ca, x0 - u, x1 - u)
                smin = np.minimum(smin, sigma_at(dxe, dye))
            inside = (u >= x0) & (u <= x1) & (v >= y0) & (v <= y1)
            smin[inside] = 0.0
            smin_all[by * NBX + bx] = smin
    return smin_all


def _block_sigma_max(G, u, v):
    """Exact max sigma over each block rectangle (convex quadratic ->
    max over the four corners)."""
    ca = -2.0 * G[0]
    cb = -G[1]
    cc = -2.0 * G[2]
    smax_all = np.zeros((NBLK, G.shape[1]))
    for by in range(NBY):
        ys = (by * BR + 0.5, by * BR + BR - 0.5)
        for bx in range(NBX):
            xs = (bx * BC + 0.5, bx * BC + BC - 0.5)
            smax = np.zeros(G.shape[1])
            for xe in xs:
                for ye in ys:
                    dx, dy = xe - u, ye - v
                    smax = np.maximum(
                        smax, 0.5 * ca * dx * dx + cb * dx * dy
                        + 0.5 * cc * dy * dy)
            smax_all[by * NBX + bx] = smax
    return smax_all


def _block_center(blk):
    by, bx = divmod(blk, NBX)
    return bx * BC + BC / 2.0, by * BR + BR / 2.0


def _pixel_basis(blk):
    """Pixel basis recentered on the block center so all basis terms stay
    small (|dx|,|dy| <= 8) and fp16 matmul inputs lose no accuracy."""
    by, bx = divmod(blk, NBX)
    cxb, cyb = _block_center(blk)
    px = np.arange(W, dtype=np.float64) + 0.5 - cxb
    py = np.arange(H, dtype=np.float64) + 0.5 - cyb
    gy, gx = np.meshgrid(py[by * BR:(by + 1) * BR],
                         px[bx * BC:(bx + 1) * BC], indexing="ij")
    fxr, fyr = gx.ravel(), gy.ravel()
    on = np.ones_like(fxr)
    return np.stack([fxr * fxr, fxr * fyr, fyr * fyr, fxr, fyr,
                     on, on], 0).astype(np.float32)   # [ROWS,128]


def _recentered_coeffs(G, idx, blk, extra_const=None):
    """Per-(block, gaussian) polynomial coefficients of m1 in block-centered
    coordinates: m1 = A dx^2 + B dxdy + C dy^2 + D dx + E dy + F."""
    cxb, cyb = _block_center(blk)
    A, B, C = G[0][idx], G[1][idx], G[2][idx]
    d_, e_, f_ = G[3][idx], G[4][idx], G[5][idx]
    if extra_const is not None:
        f_ = f_ + extra_const
    D = 2 * A * cxb + B * cyb + d_
    E = B * cxb + 2 * C * cyb + e_
    F = (A * cxb * cxb + B * cxb * cyb + C * cyb * cyb
         + d_ * cxb + e_ * cyb + f_)
    F = np.maximum(F, SENT_NEG)
    # split the constant term so the f16 matmul keeps full precision on F
    # (|F| up to 80 has f16 ulp 0.06; the PSUM accumulates hi+lo in f32)
    F_hi = F.astype(np.float16).astype(np.float64)
    F_lo = F - F_hi
    return np.stack([A, B, C, D, E, F_hi, F_lo], 0).astype(np.float32)


def _build_schedule(G, colv, op, u, v):
    T = float(os.environ.get("GS_T", "4.0"))
    QW = int(os.environ.get("GS_QW", "8"))
    smin = _block_sigma_min(G, u, v)
    cull = os.environ.get("GS_CULL", "plain")
    if cull == "w":
        # weighted: cull when op*c*e^-smin < op_max*c_max*e^-T, i.e. dimmer
        # gaussians are culled at smaller sigma
        opc = op * colv
        thr = T - np.log(opc.max() / opc)
        masks = smin < thr[None, :]
    elif cull == "o":
        # occlusion-aware: keep (front-to-back) while the contribution
        # bound op*c*e^-smin*T_ub >= opc_ref*e^-T, where T_ub is the
        # product of (1 - op*e^-smax) over kept closer gaussians (exact
        # per-block transmittance upper bound)
        smax = _block_sigma_max(G, u, v)
        opc = op * colv
        lref = np.log(opc.max())
        masks = np.zeros_like(smin, bool)
        lw = np.log(opc)
        # columns are stored back-to-front; front-to-back = reversed
        with np.errstate(divide="ignore"):
            l1a = np.log1p(-np.minimum(op * np.exp(-smax), 0.999))
        for b in range(NBLK):
            keep = masks[b]
            sb, xb, lb = smin[b], l1a[b], lw[b]
            # only gaussians passing the T_ub=1 bound can ever be kept
            cand = np.nonzero(lb - sb >= lref - T)[0]
            lT = 0.0
            for i in cand[::-1]:
                if lb[i] - sb[i] + lT >= lref - T:
                    keep[i] = True
                    lT += xb[i]
                    if lT < -T:
                        break       # saturated: nothing behind can pass
    else:
        masks = smin < T
    widths = masks.sum(1)

    nz = np.nonzero(widths)[0]
    order = nz[np.argsort(widths[nz], kind="stable")[::-1]]
    nnz = len(order)
    NR = (nnz + NCORES - 1) // NCORES          # ranks (slots per core)
    blk_of = np.full((NCORES, NR), -1, np.int64)
    for j in range(NR):
        grp = order[j * NCORES:(j + 1) * NCORES]
        if j % 2 == 1:
            grp = grp[::-1]
        # place into cores (tail rank may be short)
        if j % 2 == 1 and len(grp) < NCORES:
            blk_of[NCORES - len(grp):, j] = grp
        else:
            blk_of[:len(grp), j] = grp
    rank_w = np.zeros(NR, np.int64)
    for j in range(NR):
        bs = blk_of[:, j]
        rank_w[j] = max(widths[b] if b >= 0 else 0 for b in bs)
    slot_W = ((rank_w + 1 + QW - 1) // QW) * QW     # >=1 leading sentinel

    # cohorts: consecutive ranks sharing the same quantized width
    cohorts = []                                    # (off, j0, k, W)
    off = 0
    j = 0
    while j < NR:
        j0 = j
        Wj = int(slot_W[j])
        while j < NR and slot_W[j] == Wj:
            j += 1
        cohorts.append((off, j0, j - j0, Wj))
        off += (j - j0) * Wj
    L = off
    Lpad = (L + 511) // 512 * 512
    S = Lpad // 512

    slot_off = np.zeros(NR, np.int64)
    for (o, j0, k, Wj) in cohorts:
        for r in range(k):
            slot_off[j0 + r] = o + r * Wj

    # compression: level v scans L/2^v columns after v rounds of pair
    # combination. The column layout is permuted so pair members at every
    # level are contiguous halves: position p of a slot goes to section
    # bitreverse_v(p % 2^v) (e.g. v=2 -> [0,2,1,3], v=3 -> [0,4,2,6,1,5,3,7]).
    comp_lv = int(os.environ.get("GS_COMPRESS", "3"))
    CF = 1 << comp_lv                       # compression factor
    sec_of = np.array([int(format(r, f"0{max(comp_lv,1)}b")[::-1], 2)
                       for r in range(CF)])

    # block-diagonal matmul groups over consecutive ranks
    groups = []                                     # (j0, m)
    j = 0
    while j < NR:
        m = min(MAXROWS // ROWS, NR - j)
        groups.append((j, m))
        j += m
    grp_of = np.zeros(NR, np.int64)
    rank_in_grp = np.zeros(NR, np.int64)
    for g, (j0, m) in enumerate(groups):
        for r in range(m):
            grp_of[j0 + r] = g
            rank_in_grp[j0 + r] = r

    # segments: group-section spans intersected with the 512 grid; the tail
    # pad [L, Lpad) is appended to the last section (zero rhs -> m=0).
    # With compression the layout has CF sections of L/CF columns each and
    # slot j's section-k subrange is [k*L/CF + o/CF, k*L/CF + (o+W)/CF).
    LpC = L // CF
    spans = []
    for g, (j0, m) in enumerate(groups):
        for k in range(CF):
            a = k * LpC + int(slot_off[j0]) // CF
            b = k * LpC + int(slot_off[j0 + m - 1] + slot_W[j0 + m - 1]) // CF
            if g == len(groups) - 1 and k == CF - 1:
                b = Lpad
            spans.append((g, a, b))
    # merge adjacent spans with the same group (sections of one group tile
    # the layout contiguously), then split on the 512 PSUM-chunk grid
    spans.sort(key=lambda s: s[1])
    merged = []
    for g, a, b in spans:
        if merged and merged[-1][0] == g and merged[-1][2] == a:
            merged[-1][2] = b
        else:
            merged.append([g, a, b])
    seg_list = []
    for g, a, b in merged:
        while a < b:
            nb = min(b, (a // 512 + 1) * 512)
            seg_list.append((g, a, nb))
            a = nb
    chunk_segs = [[] for _ in range(S)]
    for g, a, b in seg_list:
        chunk_segs[a // 512].append((g, a, b))

    lcol = np.log(colv)
    mm_f16 = os.environ.get("GS_MM_DT", "f16") == "f16"

    # chunk kinds: first NCB chunks compute bt = alpha*color on a vector
    # engine (cb); the rest use a second matmul + exp (mm2). -1 = all cb.
    ncb = int(os.environ.get("GS_NCB", "-1"))
    ncb = S if ncb < 0 else min(ncb, S)

    # scan spans: merge cohorts into ~GS_NSCAN contiguous scans (in the
    # compressed domain [0, L/CF); slot j occupies [o/CF, (o+W)/CF))
    nscan = int(os.environ.get("GS_NSCAN", "12"))
    Lc = L // CF
    tgt = max(1, (Lc + nscan - 1) // max(1, nscan))
    scan_spans = []
    cur_a = 0
    for (o, j0, k, Wj) in cohorts:
        end = (o + k * Wj) // CF
        if end - cur_a >= tgt or (o, j0, k, Wj) == cohorts[-1]:
            scan_spans.append((cur_a, end))
            cur_a = end
    if cur_a < Lc:
        scan_spans.append((cur_a, Lc))

    ngroups = len(groups)
    cores = []
    for cid in range(NCORES):
        g1 = np.zeros((ROWS, Lpad), np.float32)
        g2 = np.zeros((ROWS, Lpad), np.float32)
        colr = np.zeros(Lpad, np.float32)
        ftc = np.zeros((ROWS, NR * 128), np.float32)
        for j in range(NR):
            o = int(slot_off[j])
            Wj = int(slot_W[j])
            blk = int(blk_of[cid, j])
            p = np.arange(Wj)
            dest = sec_of[p % CF] * (L // CF) + o // CF + p // CF
            if blk >= 0:
                idx = np.nonzero(masks[blk])[0]
                nb = len(idx)
                g1[:, dest[Wj - nb:]] = _recentered_coeffs(G, idx, blk)
                g2[:, dest[Wj - nb:]] = _recentered_coeffs(
                    G, idx, blk, extra_const=lcol[idx])
                colr[dest[Wj - nb:]] = colv[idx].astype(np.float32)
                ftc[:, j * 128:(j + 1) * 128] = _pixel_basis(blk)
            else:
                nb = 0
                ftc[5, j * 128:(j + 1) * 128] = 1.0
            # leading sentinels: m1 = 0 (alpha=1 -> beta=0), m2 = -80 (bt~0)
            g2[5, dest[:Wj - nb]] = SENT_NEG
        if mm_f16:
            g1 = g1.astype(np.float16)
            g2 = g2.astype(np.float16)
            ftc = ftc.astype(np.float16)
        if os.environ.get("GS_SCAN_DT", "f16") == "f16":
            colr = colr.astype(np.float16)
        cores.append({"ft": ftc, "g1": g1, "g2": g2, "colr": colr})

    plan = {
        "Lpad": Lpad, "S": S, "NR": NR, "ngroups": ngroups, "L": L,
        "ncb": ncb, "CF": CF,
        "scan_spans": tuple(scan_spans),
        "cohorts": tuple(cohorts),
        "groups": tuple(groups),
        "chunk_segs": tuple(tuple(c) for c in chunk_segs),
        "slots": tuple((int(slot_off[j]), int(slot_W[j]), int(grp_of[j]),
                        int(rank_in_grp[j])) for j in range(NR)),
        "blk_of": blk_of,
    }
    return plan, cores


# ---------------------------------------------------------------- device

def _build_module(key, plan, reps=1, loop_n=1):
    import contextlib
    import concourse.bass as bass
    import concourse.bacc as bacc
    import concourse.tile as tile
    from concourse import mybir

    f32 = mybir.dt.float32
    sdt = {"f32": mybir.dt.float32, "f16": mybir.dt.float16,
           "bf16": mybir.dt.bfloat16}[os.environ.get("GS_SCAN_DT", "f16")]
    beta_eng = os.environ.get("GS_BETA_ENG", "act")
    mul_eng = os.environ.get("GS_MUL_ENG", "dve")
    gather_eng = os.environ.get("GS_GATHER_ENG", "gps")

    Lpad, S, NR = plan["Lpad"], plan["S"], plan["NR"]
    ngroups, ncb = plan["ngroups"], plan["ncb"]
    cohorts, groups = plan["cohorts"], plan["groups"]
    chunk_segs = plan["chunk_segs"]
    scan_spans = plan["scan_spans"]
    slots = plan["slots"]
    hmax = ROWS * max(m for _, m in groups)
    skip = set(os.environ.get("GS_SKIP", "").split(","))

    mdt = (mybir.dt.float16 if os.environ.get("GS_MM_DT", "f16") == "f16"
           else f32)

    nc = bacc.Bacc(None)
    ft = nc.dram_tensor("ft", [ROWS, NR * 128], mdt, kind="ExternalInput")
    g1 = nc.dram_tensor("g1", [ROWS, Lpad], mdt, kind="ExternalInput")
    g2 = nc.dram_tensor("g2", [ROWS, Lpad], mdt, kind="ExternalInput")
    cdt = mybir.dt.float16 if sdt == mybir.dt.float16 else f32
    colr = nc.dram_tensor("colr", [Lpad], cdt, kind="ExternalInput")
    out = nc.dram_tensor("out", [128 * NR], f32, kind="ExternalOutput")

    def eng(name):
        return {"dve": nc.vector, "gps": nc.gpsimd, "act": nc.scalar}[name]

    with tile.TileContext(nc) as tc:
        with (
            tc.tile_pool(name="const", bufs=1) as consts,
            tc.tile_pool(name="big",
                         bufs=int(os.environ.get("GS_BUFS", "2"))) as big,
            tc.tile_pool(name="psum", bufs=4, space="PSUM") as psum,
        ):
            # scatter compact [6,*] inputs into block-diagonal SBUF layout
            ft_s = consts.tile([hmax, ngroups * 128], mdt)
            r1_s = consts.tile([hmax, Lpad], mdt)
            r2_s = consts.tile([hmax, Lpad], mdt)
            nc.vector.memset(r1_s[:], 0.0)
            nc.vector.memset(r2_s[:], 0.0)
            CF, L = plan["CF"], plan["L"]
            for j, (o, Wj, g, r) in enumerate(slots):
                nc.sync.dma_start(
                    out=ft_s[ROWS * r:ROWS * (r + 1), g * 128:(g + 1) * 128],
                    in_=ft[:, j * 128:(j + 1) * 128])
                for k in range(CF):
                    a = k * (L // CF) + o // CF
                    b = a + Wj // CF
                    nc.sync.dma_start(out=r1_s[ROWS * r:ROWS * (r + 1), a:b],
                                      in_=g1[:, a:b])
                    nc.sync.dma_start(out=r2_s[ROWS * r:ROWS * (r + 1), a:b],
                                      in_=g2[:, a:b])
            if ncb > 0:
                c_s = consts.tile([128, ncb * 512], cdt)
                cseg = colr[0:ncb * 512]
                bc = bass.AP(tensor=cseg.tensor, offset=cseg.offset,
                             ap=[[0, 128], cseg.ap[0]])
                nc.sync.dma_start(out=c_s[:], in_=bc)

            only = os.environ.get("GS_ONLY", "")
            Lc0 = L // plan["CF"]
            if only == "scan":
                sbc = consts.tile([128, Lc0], sdt)
                sdc = consts.tile([128, Lc0], sdt)
                nc.vector.memset(sbc[:], 0.5)
                nc.vector.memset(sdc[:], 0.01)
            elif only == "beta":
                alc = consts.tile([128, Lpad], sdt)
                nc.vector.memset(alc[:], 0.5)

            loop_cm = (
                tc.For_i(0, loop_n, 1, hint_engines=(
                    mybir.EngineType.PE, mybir.EngineType.Activation,
                    mybir.EngineType.DVE, mybir.EngineType.Pool),
                    staggered_reset=os.environ.get("GS_STAGGER", "1") == "1")
                if loop_n > 1 else contextlib.nullcontext()
            )
            Lc = L // CF
            prep_eng = os.environ.get("GS_PREP_ENG", "dve")
            with loop_cm:
                for _ in range(reps):
                    if only:
                        res = big.tile([128, NR], f32)
                        if only == "scan":
                            comp = big.tile([128, Lc], sdt)
                            for (a, b) in scan_spans:
                                nc.vector.tensor_tensor_scan(
                                    comp[:, a:b], sbc[:, a:b], sdc[:, a:b],
                                    0.0, op0=mybir.AluOpType.mult,
                                    op1=mybir.AluOpType.add)
                            for (o, j0, k, Wj) in cohorts:
                                oc, Wc = o // CF, Wj // CF
                                cv = comp[:, oc + Wc - 1:oc + Wc]
                                gap = bass.AP(
                                    tensor=cv.tensor, offset=cv.offset,
                                    ap=[cv.ap[0], [Wc, k]])
                                nc.vector.tensor_copy(res[:, j0:j0 + k], gap)
                        elif only == "mm":
                            for s in range(S):
                                ps = psum.tile([128, 1024], f32)
                                for g, a, b in chunk_segs[s]:
                                    h = ROWS * groups[g][1]
                                    lhs = ft_s[0:h, g * 128:(g + 1) * 128]
                                    nc.tensor.matmul(
                                        ps[:, a - s * 512:b - s * 512],
                                        lhsT=lhs, rhs=r1_s[0:h, a:b],
                                        start=True, stop=True)
                                    nc.tensor.matmul(
                                        ps[:, 512 + a - s * 512:
                                           512 + b - s * 512],
                                        lhsT=lhs, rhs=r2_s[0:h, a:b],
                                        start=True, stop=True)
                            nc.scalar.copy(out=res[:], in_=ps[:, 0:NR])
                        elif only == "mmwide":
                            # timing probe: one 1024-col matmul per chunk
                            # (rhs content nonsense, timing valid)
                            h = ROWS * groups[0][1]
                            lhs = ft_s[0:h, 0:128]
                            for s in range(S):
                                ps = psum.tile([128, 1024], f32)
                                w = min(1024, Lpad - s * 512)
                                nc.tensor.matmul(
                                    ps[:, 0:w], lhsT=lhs,
                                    rhs=r1_s[0:h, s * 512:s * 512 + w],
                                    start=True, stop=True)
                            nc.scalar.copy(out=res[:], in_=ps[:, 0:NR])
                        elif only == "exp":
                            ab = big.tile([128, 2 * Lpad], sdt)
                            for s in range(S):
                                ps = psum.tile([128, 1024], f32)
                                for g, a, b in chunk_segs[s]:
                                    h = ROWS * groups[g][1]
                                    lhs = ft_s[0:h, g * 128:(g + 1) * 128]
                                    nc.tensor.matmul(
                                        ps[:, a - s * 512:b - s * 512],
                                        lhsT=lhs, rhs=r1_s[0:h, a:b],
                                        start=True, stop=True)
                                    nc.tensor.matmul(
                                        ps[:, 512 + a - s * 512:
                                           512 + b - s * 512],
                                        lhsT=lhs, rhs=r2_s[0:h, a:b],
                                        start=True, stop=True)
                                nc.scalar.activation(
                                    out=ab[:, s * 512:(s + 1) * 512],
                                    in_=ps[:, 0:512],
                                    func=mybir.ActivationFunctionType.Exp,
                                    scale=1.0, bias=0.0)
                                nc.scalar.activation(
                                    out=ab[:, Lpad + s * 512:
                                           Lpad + (s + 1) * 512],
                                    in_=ps[:, 512:1024],
                                    func=mybir.ActivationFunctionType.Exp,
                                    scale=1.0, bias=0.0)
                            nc.vector.tensor_copy(res[:], ab[:, 0:NR])
                        elif only == "beta":
                            beta = big.tile([128, Lpad], sdt)
                            for s in range(S):
                                eng(beta_eng).tensor_scalar(
                                    out=beta[:, s * 512:(s + 1) * 512],
                                    in0=alc[:, s * 512:(s + 1) * 512],
                                    scalar1=-1.0, scalar2=1.0,
                                    op0=mybir.AluOpType.mult,
                                    op1=mybir.AluOpType.add)
                            nc.vector.tensor_copy(res[:], beta[:, 0:NR])
                        nc.sync.dma_start(
                            out=out[:].rearrange("(k c) -> k c", c=NR),
                            in_=res[:])
                        continue
                    ab = big.tile([128, 2 * Lpad], sdt)   # alpha | bt
                    beta = big.tile([128, Lpad], sdt)
                    comp = big.tile([128, Lc], sdt)
                    res = big.tile([128, NR], f32)
                    # chunk pairs: fused PSUM tiles, one exp per pair, all
                    # per-chunk ops trimmed at L (the tail [L, Lpad) is never
                    # read by compress/scan)
                    p = 0
                    while p < S:
                        cb = p < ncb
                        # cb chunks pair up ([m1|m1] in one 2-bank tile);
                        # mm2 chunks stay single ([m1|m2], also 2 banks) so
                        # every PSUM tile is uniformly [128, 1024]
                        np_ = min(2, ncb - p) if cb else 1
                        a0 = p * 512
                        ew = min(np_ * 512, L - a0)
                        ps = psum.tile([128, 1024], f32)
                        if "mm" not in skip:
                            for s in range(p, p + np_):
                                for g, a, b in chunk_segs[s]:
                                    b = min(b, L)
                                    if a >= b:
                                        continue
                                    h = ROWS * groups[g][1]
                                    lhs = ft_s[0:h, g * 128:(g + 1) * 128]
                                    nc.tensor.matmul(
                                        ps[:, a - a0:b - a0],
                                        lhsT=lhs, rhs=r1_s[0:h, a:b],
                                        start=True, stop=True)
                                    if not cb:
                                        nc.tensor.matmul(
                                            ps[:, ew + a - a0:ew + b - a0],
                                            lhsT=lhs, rhs=r2_s[0:h, a:b],
                                            start=True, stop=True)
                        av = ab[:, a0:a0 + ew]
                        if "exp" in skip:
                            pass
                        elif cb:
                            nc.scalar.activation(
                                out=av, in_=ps[:, 0:ew],
                                func=mybir.ActivationFunctionType.Exp,
                                scale=1.0, bias=0.0)
                        else:
                            # fused exp of [m1|m2] -> alpha at a0, bt at
                            # Lpad + a0
                            apo = bass.AP(tensor=av.tensor, offset=av.offset,
                                          ap=[av.ap[0], [Lpad, 2], [1, ew]])
                            nc.scalar.activation(
                                out=apo, in_=ps[:, 0:2 * ew],
                                func=mybir.ActivationFunctionType.Exp,
                                scale=1.0, bias=0.0)
                        p += np_
                    ncb_cols = min(ncb * 512, L)
                    if ncb_cols > 0 and "exp" not in skip:
                        eng(mul_eng).tensor_tensor(
                            out=ab[:, Lpad:Lpad + ncb_cols],
                            in0=ab[:, 0:ncb_cols],
                            in1=c_s[:, 0:ncb_cols],
                            op=mybir.AluOpType.mult)
                    if "beta" in skip:
                        pass
                    elif beta_eng == "act":
                        nc.scalar.activation(
                            out=beta[:, 0:L], in_=ab[:, 0:L],
                            func=mybir.ActivationFunctionType.Identity,
                            scale=-1.0, bias=1.0)
                    else:
                        eng(beta_eng).tensor_scalar(
                            out=beta[:, 0:L], in0=ab[:, 0:L],
                            scalar1=-1.0, scalar2=1.0,
                            op0=mybir.AluOpType.mult,
                            op1=mybir.AluOpType.add)
                    sb, sd = beta, None     # sd None -> bt lives in ab
                    half = L
                    pe = eng(prep_eng)
                    while half > Lc:
                        half //= 2
                        bn = big.tile([128, half], sdt)
                        dn = big.tile([128, half], sdt)
                        pb = sb[:, half:2 * half]
                        pd = (ab[:, Lpad + half:Lpad + 2 * half]
                              if sd is None else sd[:, half:2 * half])
                        pe.tensor_tensor(
                            out=bn[:], in0=sb[:, 0:half], in1=pb,
                            op=mybir.AluOpType.mult)
                        pe.tensor_tensor(
                            out=dn[:],
                            in0=(ab[:, Lpad:Lpad + half]
                                 if sd is None else sd[:, 0:half]),
                            in1=pb, op=mybir.AluOpType.mult)
                        pe.tensor_tensor(
                            out=dn[:], in0=dn[:], in1=pd,
                            op=mybir.AluOpType.add)
                        sb, sd = bn, dn
                    # scan spans are independent (leading sentinels reset
                    # state at slot starts) -> split across DVE and Pool to
                    # run in parallel. Greedy balance weighted by the Pool/
                    # DVE rate ratio.
                    pool_frac = float(os.environ.get("GS_SCAN_POOL", "0.0"))
                    tot_elems = sum(b - a for a, b in scan_spans)
                    pool_tgt = pool_frac * tot_elems
                    pool_acc = 0
                    for (a, b) in sorted(scan_spans,
                                         key=lambda s: s[0] - s[1]):
                        if pool_acc + (b - a) * 0.5 < pool_tgt:
                            se = nc.gpsimd
                            pool_acc += b - a
                        else:
                            se = nc.vector
                        se.tensor_tensor_scan(
                            comp[:, a:b], sb[:, a:b],
                            (ab[:, Lpad + a:Lpad + b] if sd is None
                             else sd[:, a:b]), 0.0,
                            op0=mybir.AluOpType.mult,
                            op1=mybir.AluOpType.add)
                    for (o, j0, k, Wj) in cohorts:
                        oc, Wc = o // CF, Wj // CF
                        cv = comp[:, oc + Wc - 1:oc + Wc]
                        gap = bass.AP(tensor=cv.tensor, offset=cv.offset,
                                      ap=[cv.ap[0], [Wc, k]])
                        if gather_eng == "act":
                            nc.scalar.copy(out=res[:, j0:j0 + k], in_=gap)
                        else:
                            eng(gather_eng).tensor_copy(
                                res[:, j0:j0 + k], gap)
                    nc.sync.dma_start(
                        out=out[:].rearrange("(k c) -> k c", c=NR),
                        in_=res[:])
    nc.finalize()
    return nc


# ---------------------------------------------------------------- entry

def _plan_key(plan):
    envs = tuple(os.environ.get(k, "") for k in (
        "GS_SCAN_DT", "GS_BETA_ENG", "GS_MUL_ENG", "GS_GATHER_ENG",
        "GS_SCAN_SPLIT", "GS_NCB", "GS_NSCAN", "GS_SKIP", "GS_ACT",
        "GS_COMPRESS", "GS_PREP_ENG", "GS_STAGGER", "GS_ONLY", "GS_MM_DT",
        "GS_MUL_GPS", "GS_SCAN_POOL", "GS_BUFS"))
    return (plan["Lpad"], plan["NR"], plan["cohorts"], plan["groups"],
            plan["chunk_segs"], plan["ncb"], plan["scan_spans"], envs)


def _prepare(inputs, reps=1, loop_n=1):
    reps = max(reps, int(os.environ.get("GS_REPS", "1")))
    # unroll: run U renders per hardware-loop iteration so tile-pool double
    # buffering pipelines consecutive renders (loop_n is divided to keep the
    # total render count)
    U = int(os.environ.get("GS_UNROLL", "8"))
    if loop_n > 1 and U > 1:
        loop_n = max(1, loop_n // U)
        reps = reps * U
    G, colv, op, u, v = _preprocess(**inputs)
    plan, cores = _build_schedule(G, colv, op, u, v)
    key = (_plan_key(plan), reps, loop_n)
    if key not in _cache:
        _cache[key] = _build_module(key, plan, reps=reps, loop_n=loop_n)
    nc = _cache[key]
    in_maps = [{k: cores[cid][k] for k in ("ft", "g1", "g2", "colr")}
               for cid in range(NCORES)]
    return nc, in_maps, plan


def _assemble(results, plan):
    img = np.zeros((H, W), np.float32)
    blk_of = plan["blk_of"]
    NR = plan["NR"]
    for cid in range(NCORES):
        res = results[cid]["out"].reshape(128, NR)
        for j in range(NR):
            blk = int(blk_of[cid, j])
            if blk < 0:
                continue
            by, bx = divmod(blk, NBX)
            img[by * BR:(by + 1) * BR, bx * BC:(bx + 1) * BC] = (
                res[:, j].reshape(BR, BC))
    return img.reshape(1, 1, H, W)


def kernel(**inputs):
    from concourse.bass_utils import run_bass_kernel_spmd

    inputs = {k: np.asarray(v) for k, v in inputs.items()}
    nc, in_maps, plan = _prepare(inputs)
    res = run_bass_kernel_spmd(nc, in_maps, core_ids=list(range(NCORES)))
    return _assemble(res.results, plan)



# revision 30
# speedup vs baseline: 1.0349x; 1.0349x over previous
"""Trainium2 Bass kernel for 2D Gaussian Splatting (N=1024, 256x256, 8 cores).

Math: sigma[p,i] is quadratic in pixel coords, so m1 = log(op) - sigma is
a matmul ft[6m,128pix]^T @ g[6m,cols] per 128-pixel block. Consecutive
blocks are merged into block-diagonal groups (contraction 6m <= 126; PE
cost depends only on streamed columns), and coordinates are recentered per
block so every term stays small and fp16 matmul inputs lose no accuracy
(fp32 PE matmul is ~2x slower). alpha = exp(m1) on ACT, bt = alpha*color
on DVE, beta = 1-alpha on ACT (Identity, scale=-1, bias=1); front-to-back
compositing is evaluated back-to-front as the affine scan
C = beta*C + bt along the gaussian axis.

The scan runs ~3.4 cyc/elem on DVE with no 16-bit speedup, so columns are
pre-combined 3 levels (pairs -> quads -> octs) with cheap f16 2x
tensor_tensor ops and the scan covers only L/8 columns. The column layout
is permuted so that pair members at every level are contiguous halves:
position p of a slot goes to section bitreverse3(p % 8).

Culling: 512 blocks of 8x16 pixels; a (gaussian, block) pair is kept iff
the exact minimal sigma over the block rectangle is < 5 (~3.3e-3 image
rel err vs the 2e-2 budget). Non-empty blocks are snake-dealt by width
rank onto the 8 cores (SPMD: identical program, per-core data). Slot
widths are quantized to multiples of 8 and grouped into uniform-width
cohorts; leading sentinel columns (m1=0 -> beta=0) reset the scan state
at every slot start, and one strided copy per cohort gathers the slot
finals.

The For_i timing loop uses staggered_reset (no all-engine barrier per
iteration) and unrolls 8 renders per iteration so tile-pool double
buffering pipelines consecutive renders. Inputs ship compact ([6,*] fp16
coefficient tensors) and are scattered into the block-diagonal SBUF
layout by one-time DMAs. The host reassembles the image from per-core
slot outputs; fully-culled blocks render as zero.
"""

import os
import math
import numpy as np

H = 256
W = 256
N = 1024
NCORES = 8
BR, BC = 8, 16                 # block = 8 rows x 16 cols = 128 pixels
NBY, NBX = H // BR, W // BC
NBLK = NBY * NBX               # 512
SENT_NEG = -80.0
EPS2D = 0.3
ROWS = 7                       # basis rows: x2 xy y2 x y 1 1 (F split hi/lo)
MAXROWS = 126                  # max contraction rows per merged matmul

_cache = {}


# ---------------------------------------------------------------- host math

def _preprocess(means, quats, scales, rgbs, opacities, viewmat, K):
    """Float64 per-gaussian preprocessing. Returns (in back-to-front order):
    G6 [6,N] basis coefficients of log(op)-sigma and colors [N]."""
    md = means.astype(np.float64)
    Rv = viewmat[:3, :3].astype(np.float64)
    t = viewmat[:3, 3].astype(np.float64)
    p_cam = md @ Rv.T + t
    x, y, z = p_cam[:, 0], p_cam[:, 1], p_cam[:, 2]
    fx, fy = float(K[0, 0]), float(K[1, 1])
    cx, cy = float(K[0, 2]), float(K[1, 2])
    inv_z = 1.0 / z
    u = fx * x * inv_z + cx
    v = fy * y * inv_z + cy

    th = quats.astype(np.float64)
    ct, st = np.cos(th), np.sin(th)
    zr, on = np.zeros_like(ct), np.ones_like(ct)
    R3 = np.stack([np.stack([ct, -st, zr], -1),
                   np.stack([st, ct, zr], -1),
                   np.stack([zr, zr, on], -1)], -2)
    M = R3 * scales.astype(np.float64)[:, None, :]
    cov3 = M @ np.swapaxes(M, -1, -2)
    cov_cam = np.einsum('ij,njk,lk->nil', Rv, cov3, Rv)
    j0 = np.stack([fx * inv_z, zr, -fx * x * inv_z * inv_z], -1)
    j1 = np.stack([zr, fy * inv_z, -fy * y * inv_z * inv_z], -1)
    J = np.stack([j0, j1], -2)
    cov2 = np.einsum('nij,njk,nlk->nil', J, cov_cam, J)
    a = cov2[:, 0, 0] + EPS2D
    b = cov2[:, 0, 1]
    c = cov2[:, 1, 1] + EPS2D
    det = a * c - b * b
    ca, cb, cc = c / det, -b / det, a / det

    op = 1.0 / (1.0 + np.exp(-opacities.astype(np.float64)))
    colv = 1.0 / (1.0 + np.exp(-rgbs.astype(np.float64)[:, 0]))

    # reference sorts by fp32 camera z ascending (stable); we composite
    # back-to-front = exact reverse
    order = np.argsort(z.astype(np.float32), kind="stable")
    rev = order[::-1]

    ca2, cc2 = 0.5 * ca, 0.5 * cc
    lop = np.log(op)
    d = -(ca * u + cb * v)
    e = -(cb * u + cc * v)
    f = ca2 * u * u + cb * u * v + cc2 * v * v
    G = np.stack([-ca2, -cb, -cc2, -d, -e, lop - f], 0)[:, rev]  # [6,N] f64
    return G, colv[rev], op[rev], u[rev], v[rev]


def _block_sigma_min(G, u, v):
    """Exact minimal sigma over each block rectangle: 0 if the center is
    inside, else the min over the four edges (1D quadratic, clamped)."""
    ca = -2.0 * G[0]
    cb = -G[1]
    cc = -2.0 * G[2]

    def sigma_at(dx, dy):
        return 0.5 * ca * dx * dx + cb * dx * dy + 0.5 * cc * dy * dy

    smin_all = np.zeros((NBLK, G.shape[1]))
    for by in range(NBY):
        y0, y1 = by * BR + 0.5, by * BR + BR - 0.5
        for bx in range(NBX):
            x0, x1 = bx * BC + 0.5, bx * BC + BC - 0.5
            smin = np.full(G.shape[1], np.inf)
            for xe in (x0, x1):
                dxe = xe - u
                dye = np.clip(-cb * dxe / cc, y0 - v, y1 - v)
                smin = np.minimum(smin, sigma_at(dxe, dye))
            for ye in (y0, y1):
                dye = ye - v
                dxe = np.clip(-cb * dye / ca, x0 - u, x1 - u)
                smin = np.minimum(smin, sigma_at(dxe, dye))
            inside = (u >= x0) & (u <= x1) & (v >= y0) & (v <= y1)
            smin[inside] = 0.0
            smin_all[by * NBX + bx] = smin
    return smin_all


def _rect_sigma_minmax(G, u, v, x0, x1, y0, y1):
    """Exact min and max sigma over a pixel rectangle: min via clamped 1D
    quadratics on the edges (0 if center inside), max over the corners."""
    ca = -2.0 * G[0]
    cb = -G[1]
    cc = -2.0 * G[2]

    def s_at(dx, dy):
        return 0.5 * ca * dx * dx + cb * dx * dy + 0.5 * cc * dy * dy

    smin = np.full(G.shape[1], np.inf)
    smax = np.zeros(G.shape[1])
    for xe in (x0, x1):
        dxe = xe - u
        dye = np.clip(-cb * dxe / cc, y0 - v, y1 - v)
        smin = np.minimum(smin, s_at(dxe, dye))
    for ye in (y0, y1):
        dye = ye - v
        dxe = np.clip(-cb * dye / ca, x0 - u, x1 - u)
        smin = np.minimum(smin, s_at(dxe, dye))
    inside = (u >= x0) & (u <= x1) & (v >= y0) & (v <= y1)
    smin[inside] = 0.0
    for xe in (x0, x1):
        for ye in (y0, y1):
            smax = np.maximum(smax, s_at(xe - u, ye - v))
    return smin, smax


def _block_center(blk):
    by, bx = divmod(blk, NBX)
    return bx * BC + BC / 2.0, by * BR + BR / 2.0


def _pixel_basis(blk):
    """Pixel basis recentered on the block center so all basis terms stay
    small (|dx|,|dy| <= 8) and fp16 matmul inputs lose no accuracy."""
    by, bx = divmod(blk, NBX)
    cxb, cyb = _block_center(blk)
    px = np.arange(W, dtype=np.float64) + 0.5 - cxb
    py = np.arange(H, dtype=np.float64) + 0.5 - cyb
    gy, gx = np.meshgrid(py[by * BR:(by + 1) * BR],
                         px[bx * BC:(bx + 1) * BC], indexing="ij")
    fxr, fyr = gx.ravel(), gy.ravel()
    on = np.ones_like(fxr)
    return np.stack([fxr * fxr, fxr * fyr, fyr * fyr, fxr, fyr,
                     on, on], 0).astype(np.float32)   # [ROWS,128]


def _recentered_coeffs(G, idx, blk, extra_const=None):
    """Per-(block, gaussian) polynomial coefficients of m1 in block-centered
    coordinates: m1 = A dx^2 + B dxdy + C dy^2 + D dx + E dy + F."""
    cxb, cyb = _block_center(blk)
    A, B, C = G[0][idx], G[1][idx], G[2][idx]
    d_, e_, f_ = G[3][idx], G[4][idx], G[5][idx]
    if extra_const is not None:
        f_ = f_ + extra_const
    D = 2 * A * cxb + B * cyb + d_
    E = B * cxb + 2 * C * cyb + e_
    F = (A * cxb * cxb + B * cxb * cyb + C * cyb * cyb
         + d_ * cxb + e_ * cyb + f_)
    F = np.maximum(F, SENT_NEG)
    # split the constant term so the f16 matmul keeps full precision on F
    # (|F| up to 80 has f16 ulp 0.06; the PSUM accumulates hi+lo in f32)
    F_hi = F.astype(np.float16).astype(np.float64)
    F_lo = F - F_hi
    return np.stack([A, B, C, D, E, F_hi, F_lo], 0).astype(np.float32)


def _build_schedule(G, colv, op, u, v):
    T = float(os.environ.get("GS_T", "4.0"))
    QW = int(os.environ.get("GS_QW", "8"))
    smin = _block_sigma_min(G, u, v)
    cull = os.environ.get("GS_CULL", "plain")
    if cull == "w":
        # weighted: cull when op*c*e^-smin < op_max*c_max*e^-T, i.e. dimmer
        # gaussians are culled at smaller sigma
        opc = op * colv
        thr = T - np.log(opc.max() / opc)
        masks = smin < thr[None, :]
    elif cull == "o":
        # occlusion-aware at sub-block resolution: keep (front-to-back)
        # iff for SOME sub-rectangle r the contribution bound
        # op*c*e^-smin_r * T_ub_r >= opc_ref*e^-T holds, where T_ub_r is
        # the product of (1 - op*e^-smax_r) over kept closer gaussians
        OS = int(os.environ.get("GS_OSUB", "2"))
        NG = G.shape[1]
        RR = OS * OS
        smin_sub = np.zeros((NBLK, RR, NG))
        smax_sub = np.zeros((NBLK, RR, NG))
        for by in range(NBY):
            for bx in range(NBX):
                b = by * NBX + bx
                r = 0
                for sy in range(OS):
                    y0 = by * BR + sy * BR // OS + 0.5
                    y1 = by * BR + (sy + 1) * BR // OS - 0.5
                    for sx in range(OS):
                        x0 = bx * BC + sx * BC // OS + 0.5
                        x1 = bx * BC + (sx + 1) * BC // OS - 0.5
                        smin_sub[b, r], smax_sub[b, r] = _rect_sigma_minmax(
                            G, u, v, x0, x1, y0, y1)
                        r += 1
        opc = op * colv
        lref = np.log(opc.max())
        lw = np.log(opc)
        masks = np.zeros_like(smin, bool)
        with np.errstate(divide="ignore"):
            l1a = np.log1p(-np.minimum(op * np.exp(-smax_sub), 0.999))
        for b in range(NBLK):
            keep = masks[b]
            sb = smin_sub[b]            # [RR, NG]
            xb = l1a[b]
            cand = np.nonzero(lw - sb.min(0) >= lref - T)[0]
            lT = np.zeros(RR)
            for i in cand[::-1]:
                if np.any(lw[i] - sb[:, i] + lT >= lref - T):
                    keep[i] = True
                    lT += xb[:, i]
                    if lT.max() < -T:
                        break       # saturated: nothing behind can pass
    else:
        masks = smin < T
    widths = masks.sum(1)

    nz = np.nonzero(widths)[0]
    order = nz[np.argsort(widths[nz], kind="stable")[::-1]]
    nnz = len(order)
    NR = (nnz + NCORES - 1) // NCORES          # ranks (slots per core)
    blk_of = np.full((NCORES, NR), -1, np.int64)
    for j in range(NR):
        grp = order[j * NCORES:(j + 1) * NCORES]
        if j % 2 == 1:
            grp = grp[::-1]
        # place into cores (tail rank may be short)
        if j % 2 == 1 and len(grp) < NCORES:
            blk_of[NCORES - len(grp):, j] = grp
        else:
            blk_of[:len(grp), j] = grp
    rank_w = np.zeros(NR, np.int64)
    for j in range(NR):
        bs = blk_of[:, j]
        rank_w[j] = max(widths[b] if b >= 0 else 0 for b in bs)
    slot_W = ((rank_w + 1 + QW - 1) // QW) * QW     # >=1 leading sentinel

    # cohorts: consecutive ranks sharing the same quantized width
    cohorts = []                                    # (off, j0, k, W)
    off = 0
    j = 0
    while j < NR:
        j0 = j
        Wj = int(slot_W[j])
        while j < NR and slot_W[j] == Wj:
            j += 1
        cohorts.append((off, j0, j - j0, Wj))
        off += (j - j0) * Wj
    L = off
    Lpad = (L + 511) // 512 * 512
    S = Lpad // 512

    slot_off = np.zeros(NR, np.int64)
    for (o, j0, k, Wj) in cohorts:
        for r in range(k):
            slot_off[j0 + r] = o + r * Wj

    # compression: level v scans L/2^v columns after v rounds of pair
    # combination. The column layout is permuted so pair members at every
    # level are contiguous halves: position p of a slot goes to section
    # bitreverse_v(p % 2^v) (e.g. v=2 -> [0,2,1,3], v=3 -> [0,4,2,6,1,5,3,7]).
    comp_lv = int(os.environ.get("GS_COMPRESS", "3"))
    CF = 1 << comp_lv                       # compression factor
    sec_of = np.array([int(format(r, f"0{max(comp_lv,1)}b")[::-1], 2)
                       for r in range(CF)])

    # block-diagonal matmul groups over consecutive ranks
    groups = []                                     # (j0, m)
    j = 0
    while j < NR:
        m = min(MAXROWS // ROWS, NR - j)
        groups.append((j, m))
        j += m
    grp_of = np.zeros(NR, np.int64)
    rank_in_grp = np.zeros(NR, np.int64)
    for g, (j0, m) in enumerate(groups):
        for r in range(m):
            grp_of[j0 + r] = g
            rank_in_grp[j0 + r] = r

    # segments: group-section spans intersected with the 512 grid; the tail
    # pad [L, Lpad) is appended to the last section (zero rhs -> m=0).
    # With compression the layout has CF sections of L/CF columns each and
    # slot j's section-k subrange is [k*L/CF + o/CF, k*L/CF + (o+W)/CF).
    LpC = L // CF
    spans = []
    for g, (j0, m) in enumerate(groups):
        for k in range(CF):
            a = k * LpC + int(slot_off[j0]) // CF
            b = k * LpC + int(slot_off[j0 + m - 1] + slot_W[j0 + m - 1]) // CF
            if g == len(groups) - 1 and k == CF - 1:
                b = Lpad
            spans.append((g, a, b))
    # merge adjacent spans with the same group (sections of one group tile
    # the layout contiguously), then split on the 512 PSUM-chunk grid
    spans.sort(key=lambda s: s[1])
    merged = []
    for g, a, b in spans:
        if merged and merged[-1][0] == g and merged[-1][2] == a:
            merged[-1][2] = b
        else:
            merged.append([g, a, b])
    seg_list = []
    for g, a, b in merged:
        while a < b:
            nb = min(b, (a // 512 + 1) * 512)
            seg_list.append((g, a, nb))
            a = nb
    chunk_segs = [[] for _ in range(S)]
    for g, a, b in seg_list:
        chunk_segs[a // 512].append((g, a, b))

    lcol = np.log(colv)
    mm_f16 = os.environ.get("GS_MM_DT", "f16") == "f16"

    # chunk kinds: first NCB chunks compute bt = alpha*color on a vector
    # engine (cb); the rest use a second matmul + exp (mm2). -1 = all cb.
    ncb = int(os.environ.get("GS_NCB", "-1"))
    ncb = S if ncb < 0 else min(ncb, S)

    # scan spans: merge cohorts into ~GS_NSCAN contiguous scans (in the
    # compressed domain [0, L/CF); slot j occupies [o/CF, (o+W)/CF))
    nscan = int(os.environ.get("GS_NSCAN", "12"))
    Lc = L // CF
    tgt = max(1, (Lc + nscan - 1) // max(1, nscan))
    scan_spans = []
    cur_a = 0
    for (o, j0, k, Wj) in cohorts:
        end = (o + k * Wj) // CF
        if end - cur_a >= tgt or (o, j0, k, Wj) == cohorts[-1]:
            scan_spans.append((cur_a, end))
            cur_a = end
    if cur_a < Lc:
        scan_spans.append((cur_a, Lc))

    ngroups = len(groups)
    cores = []
    for cid in range(NCORES):
        g1 = np.zeros((ROWS, Lpad), np.float32)
        g2 = np.zeros((ROWS, Lpad), np.float32)
        colr = np.zeros(Lpad, np.float32)
        ftc = np.zeros((ROWS, NR * 128), np.float32)
        for j in range(NR):
            o = int(slot_off[j])
            Wj = int(slot_W[j])
            blk = int(blk_of[cid, j])
            p = np.arange(Wj)
            dest = sec_of[p % CF] * (L // CF) + o // CF + p // CF
            if blk >= 0:
                idx = np.nonzero(masks[blk])[0]
                nb = len(idx)
                g1[:, dest[Wj - nb:]] = _recentered_coeffs(G, idx, blk)
                g2[:, dest[Wj - nb:]] = _recentered_coeffs(
                    G, idx, blk, extra_const=lcol[idx])
                colr[dest[Wj - nb:]] = colv[idx].astype(np.float32)
                ftc[:, j * 128:(j + 1) * 128] = _pixel_basis(blk)
            else:
                nb = 0
                ftc[5, j * 128:(j + 1) * 128] = 1.0
            # leading sentinels: m1 = 0 (alpha=1 -> beta=0), m2 = -80 (bt~0)
            g2[5, dest[:Wj - nb]] = SENT_NEG
        if mm_f16:
            g1 = g1.astype(np.float16)
            g2 = g2.astype(np.float16)
            ftc = ftc.astype(np.float16)
        if os.environ.get("GS_SCAN_DT", "f16") == "f16":
            colr = colr.astype(np.float16)
        cores.append({"ft": ftc, "g1": g1, "g2": g2, "colr": colr})

    plan = {
        "Lpad": Lpad, "S": S, "NR": NR, "ngroups": ngroups, "L": L,
        "ncb": ncb, "CF": CF,
        "scan_spans": tuple(scan_spans),
        "cohorts": tuple(cohorts),
        "groups": tuple(groups),
        "chunk_segs": tuple(tuple(c) for c in chunk_segs),
        "slots": tuple((int(slot_off[j]), int(slot_W[j]), int(grp_of[j]),
                        int(rank_in_grp[j])) for j in range(NR)),
        "blk_of": blk_of,
    }
    return plan, cores


# ---------------------------------------------------------------- device

def _build_module(key, plan, reps=1, loop_n=1):
    import contextlib
    import concourse.bass as bass
    import concourse.bacc as bacc
    import concourse.tile as tile
    from concourse import mybir

    f32 = mybir.dt.float32
    sdt = {"f32": mybir.dt.float32, "f16": mybir.dt.float16,
           "bf16": mybir.dt.bfloat16}[os.environ.get("GS_SCAN_DT", "f16")]
    beta_eng = os.environ.get("GS_BETA_ENG", "act")
    mul_eng = os.environ.get("GS_MUL_ENG", "dve")
    gather_eng = os.environ.get("GS_GATHER_ENG", "gps")

    Lpad, S, NR = plan["Lpad"], plan["S"], plan["NR"]
    ngroups, ncb = plan["ngroups"], plan["ncb"]
    cohorts, groups = plan["cohorts"], plan["groups"]
    chunk_segs = plan["chunk_segs"]
    scan_spans = plan["scan_spans"]
    slots = plan["slots"]
    hmax = ROWS * max(m for _, m in groups)
    skip = set(os.environ.get("GS_SKIP", "").split(","))

    mdt = (mybir.dt.float16 if os.environ.get("GS_MM_DT", "f16") == "f16"
           else f32)

    use_apg = (gather_eng == "apg" and plan["CF"] == 1 and NR <= 16)
    nc = bacc.Bacc(None)
    ft = nc.dram_tensor("ft", [ROWS, NR * 128], mdt, kind="ExternalInput")
    if use_apg:
        gidx = nc.dram_tensor("gidx", [128, 1], mybir.dt.int16,
                              kind="ExternalInput")
    g1 = nc.dram_tensor("g1", [ROWS, Lpad], mdt, kind="ExternalInput")
    g2 = nc.dram_tensor("g2", [ROWS, Lpad], mdt, kind="ExternalInput")
    cdt = mybir.dt.float16 if sdt == mybir.dt.float16 else f32
    colr = nc.dram_tensor("colr", [Lpad], cdt, kind="ExternalInput")
    out = nc.dram_tensor("out", [128 * NR], f32, kind="ExternalOutput")

    def eng(name):
        return {"dve": nc.vector, "gps": nc.gpsimd, "act": nc.scalar}[name]

    with tile.TileContext(nc) as tc:
        with (
            tc.tile_pool(name="const", bufs=1) as consts,
            tc.tile_pool(name="big",
                         bufs=int(os.environ.get("GS_BUFS", "2"))) as big,
            tc.tile_pool(name="psum", bufs=4, space="PSUM") as psum,
        ):
            # scatter compact [6,*] inputs into block-diagonal SBUF layout
            ft_s = consts.tile([hmax, ngroups * 128], mdt)
            r1_s = consts.tile([hmax, Lpad], mdt)
            r2_s = consts.tile([hmax, Lpad], mdt)
            nc.vector.memset(r1_s[:], 0.0)
            nc.vector.memset(r2_s[:], 0.0)
            CF, L = plan["CF"], plan["L"]
            for j, (o, Wj, g, r) in enumerate(slots):
                nc.sync.dma_start(
                    out=ft_s[ROWS * r:ROWS * (r + 1), g * 128:(g + 1) * 128],
                    in_=ft[:, j * 128:(j + 1) * 128])
                for k in range(CF):
                    a = k * (L // CF) + o // CF
                    b = a + Wj // CF
                    nc.sync.dma_start(out=r1_s[ROWS * r:ROWS * (r + 1), a:b],
                                      in_=g1[:, a:b])
                    nc.sync.dma_start(out=r2_s[ROWS * r:ROWS * (r + 1), a:b],
                                      in_=g2[:, a:b])
            if ncb > 0:
                c_s = consts.tile([128, ncb * 512], cdt)
                cseg = colr[0:ncb * 512]
                bc = bass.AP(tensor=cseg.tensor, offset=cseg.offset,
                             ap=[[0, 128], cseg.ap[0]])
                nc.sync.dma_start(out=c_s[:], in_=bc)
            if use_apg:
                gidx_s = consts.tile([128, 1], mybir.dt.int16)
                nc.sync.dma_start(out=gidx_s[:], in_=gidx[:])

            only = os.environ.get("GS_ONLY", "")
            Lc0 = L // plan["CF"]
            if only == "scan":
                sbc = consts.tile([128, Lc0], sdt)
                sdc = consts.tile([128, Lc0], sdt)
                nc.vector.memset(sbc[:], 0.5)
                nc.vector.memset(sdc[:], 0.01)
            elif only == "beta":
                alc = consts.tile([128, Lpad], sdt)
                nc.vector.memset(alc[:], 0.5)

            loop_cm = (
                tc.For_i(0, loop_n, 1, hint_engines=(
                    mybir.EngineType.PE, mybir.EngineType.Activation,
                    mybir.EngineType.DVE, mybir.EngineType.Pool),
                    staggered_reset=os.environ.get("GS_STAGGER", "1") == "1")
                if loop_n > 1 else contextlib.nullcontext()
            )
            Lc = L // CF
            prep_eng = os.environ.get("GS_PREP_ENG", "dve")
            with loop_cm:
                for _ in range(reps):
                    if only:
                        res = big.tile([128, NR], f32)
                        if only == "scan":
                            comp = big.tile([128, Lc], sdt)
                            for (a, b) in scan_spans:
                                nc.vector.tensor_tensor_scan(
                                    comp[:, a:b], sbc[:, a:b], sdc[:, a:b],
                                    0.0, op0=mybir.AluOpType.mult,
                                    op1=mybir.AluOpType.add)
                            for (o, j0, k, Wj) in cohorts:
                                oc, Wc = o // CF, Wj // CF
                                cv = comp[:, oc + Wc - 1:oc + Wc]
                                gap = bass.AP(
                                    tensor=cv.tensor, offset=cv.offset,
                                    ap=[cv.ap[0], [Wc, k]])
                                nc.vector.tensor_copy(res[:, j0:j0 + k], gap)
                        elif only == "mm":
                            for s in range(S):
                                ps = psum.tile([128, 1024], f32)
                                for g, a, b in chunk_segs[s]:
                                    h = ROWS * groups[g][1]
                                    lhs = ft_s[0:h, g * 128:(g + 1) * 128]
                                    nc.tensor.matmul(
                                        ps[:, a - s * 512:b - s * 512],
                                        lhsT=lhs, rhs=r1_s[0:h, a:b],
                                        start=True, stop=True)
                                    nc.tensor.matmul(
                                        ps[:, 512 + a - s * 512:
                                           512 + b - s * 512],
                                        lhsT=lhs, rhs=r2_s[0:h, a:b],
                                        start=True, stop=True)
                            nc.scalar.copy(out=res[:], in_=ps[:, 0:NR])
                        elif only == "mmwide":
                            # timing probe: one 1024-col matmul per chunk
                            # (rhs content nonsense, timing valid)
                            h = ROWS * groups[0][1]
                            lhs = ft_s[0:h, 0:128]
                            for s in range(S):
                                ps = psum.tile([128, 1024], f32)
                                w = min(1024, Lpad - s * 512)
                                nc.tensor.matmul(
                                    ps[:, 0:w], lhsT=lhs,
                                    rhs=r1_s[0:h, s * 512:s * 512 + w],
                                    start=True, stop=True)
                            nc.scalar.copy(out=res[:], in_=ps[:, 0:NR])
                        elif only == "exp":
                            ab = big.tile([128, 2 * Lpad], sdt)
                            for s in range(S):
                                ps = psum.tile([128, 1024], f32)
                                for g, a, b in chunk_segs[s]:
                                    h = ROWS * groups[g][1]
                                    lhs = ft_s[0:h, g * 128:(g + 1) * 128]
                                    nc.tensor.matmul(
                                        ps[:, a - s * 512:b - s * 512],
                                        lhsT=lhs, rhs=r1_s[0:h, a:b],
                                        start=True, stop=True)
                                    nc.tensor.matmul(
                                        ps[:, 512 + a - s * 512:
                                           512 + b - s * 512],
                                        lhsT=lhs, rhs=r2_s[0:h, a:b],
                                        start=True, stop=True)
                                nc.scalar.activation(
                                    out=ab[:, s * 512:(s + 1) * 512],
                                    in_=ps[:, 0:512],
                                    func=mybir.ActivationFunctionType.Exp,
                                    scale=1.0, bias=0.0)
                                nc.scalar.activation(
                                    out=ab[:, Lpad + s * 512:
                                           Lpad + (s + 1) * 512],
                                    in_=ps[:, 512:1024],
                                    func=mybir.ActivationFunctionType.Exp,
                                    scale=1.0, bias=0.0)
                            nc.vector.tensor_copy(res[:], ab[:, 0:NR])
                        elif only == "beta":
                            beta = big.tile([128, Lpad], sdt)
                            for s in range(S):
                                eng(beta_eng).tensor_scalar(
                                    out=beta[:, s * 512:(s + 1) * 512],
                                    in0=alc[:, s * 512:(s + 1) * 512],
                                    scalar1=-1.0, scalar2=1.0,
                                    op0=mybir.AluOpType.mult,
                                    op1=mybir.AluOpType.add)
                            nc.vector.tensor_copy(res[:], beta[:, 0:NR])
                        nc.sync.dma_start(
                            out=out[:].rearrange("(k c) -> k c", c=NR),
                            in_=res[:])
                        continue
                    ab = big.tile([128, 2 * Lpad], sdt)   # alpha | bt
                    beta = big.tile([128, Lpad], sdt)
                    comp = big.tile([128, Lc], sdt)
                    res = big.tile([128, NR], f32)
                    # chunk pairs: fused PSUM tiles, one exp per pair, all
                    # per-chunk ops trimmed at L (the tail [L, Lpad) is never
                    # read by compress/scan)
                    p = 0
                    while p < S:
                        cb = p < ncb
                        # cb chunks pair up ([m1|m1] in one 2-bank tile);
                        # mm2 chunks stay single ([m1|m2], also 2 banks) so
                        # every PSUM tile is uniformly [128, 1024]
                        np_ = min(2, ncb - p) if cb else 1
                        a0 = p * 512
                        ew = min(np_ * 512, L - a0)
                        ps = psum.tile([128, 1024], f32)
                        if "mm" not in skip:
                            for s in range(p, p + np_):
                                for g, a, b in chunk_segs[s]:
                                    b = min(b, L)
                                    if a >= b:
                                        continue
                                    h = ROWS * groups[g][1]
                                    lhs = ft_s[0:h, g * 128:(g + 1) * 128]
                                    nc.tensor.matmul(
                                        ps[:, a - a0:b - a0],
                                        lhsT=lhs, rhs=r1_s[0:h, a:b],
                                        start=True, stop=True)
                                    if not cb:
                                        # m2 at bank-aligned offset 512
                                        # (matmul out must not cross a
                                        # PSUM bank boundary)
                                        nc.tensor.matmul(
                                            ps[:, 512 + a - a0:512 + b - a0],
                                            lhsT=lhs, rhs=r2_s[0:h, a:b],
                                            start=True, stop=True)
                        av = ab[:, a0:a0 + ew]
                        if "exp" in skip:
                            pass
                        elif cb:
                            nc.scalar.activation(
                                out=av, in_=ps[:, 0:ew],
                                func=mybir.ActivationFunctionType.Exp,
                                scale=1.0, bias=0.0)
                        else:
                            # [m1|m2] -> alpha at a0, bt at Lpad + a0. Two
                            # activations with plain tile-slice outputs: a
                            # single strided-AP write is invisible to the
                            # tile dependency tracker (races under bufs=2)
                            nc.scalar.activation(
                                out=av, in_=ps[:, 0:ew],
                                func=mybir.ActivationFunctionType.Exp,
                                scale=1.0, bias=0.0)
                            nc.scalar.activation(
                                out=ab[:, Lpad + a0:Lpad + a0 + ew],
                                in_=ps[:, 512:512 + ew],
                                func=mybir.ActivationFunctionType.Exp,
                                scale=1.0, bias=0.0)
                        p += np_
                    ncb_cols = min(ncb * 512, L)
                    if ncb_cols > 0 and "exp" not in skip:
                        # bt = alpha * color, optionally split DVE / Pool
                        mk = int(float(os.environ.get("GS_MUL_POOL", "0"))
                                 * ncb_cols)
                        if mk > 0:
                            nc.gpsimd.tensor_tensor(
                                out=ab[:, Lpad:Lpad + mk],
                                in0=ab[:, 0:mk], in1=c_s[:, 0:mk],
                                op=mybir.AluOpType.mult)
                        if mk < ncb_cols:
                            eng(mul_eng).tensor_tensor(
                                out=ab[:, Lpad + mk:Lpad + ncb_cols],
                                in0=ab[:, mk:ncb_cols],
                                in1=c_s[:, mk:ncb_cols],
                                op=mybir.AluOpType.mult)
                    if "beta" not in skip:
                        # beta = 1 - alpha, per chunk so scan spans can
                        # start as soon as their range is ready
                        for s in range(S):
                            b0, b1 = s * 512, min((s + 1) * 512, L)
                            if b0 >= b1:
                                continue
                            if beta_eng == "act":
                                nc.scalar.activation(
                                    out=beta[:, b0:b1], in_=ab[:, b0:b1],
                                    func=(mybir.ActivationFunctionType
                                          .Identity),
                                    scale=-1.0, bias=1.0)
                            else:
                                eng(beta_eng).tensor_scalar(
                                    out=beta[:, b0:b1], in0=ab[:, b0:b1],
                                    scalar1=-1.0, scalar2=1.0,
                                    op0=mybir.AluOpType.mult,
                                    op1=mybir.AluOpType.add)
                    sb, sd = beta, None     # sd None -> bt lives in ab
                    half = L
                    pe = eng(prep_eng)
                    while half > Lc:
                        half //= 2
                        bn = big.tile([128, half], sdt)
                        dn = big.tile([128, half], sdt)
                        pb = sb[:, half:2 * half]
                        pd = (ab[:, Lpad + half:Lpad + 2 * half]
                              if sd is None else sd[:, half:2 * half])
                        pe.tensor_tensor(
                            out=bn[:], in0=sb[:, 0:half], in1=pb,
                            op=mybir.AluOpType.mult)
                        pe.tensor_tensor(
                            out=dn[:],
                            in0=(ab[:, Lpad:Lpad + half]
                                 if sd is None else sd[:, 0:half]),
                            in1=pb, op=mybir.AluOpType.mult)
                        pe.tensor_tensor(
                            out=dn[:], in0=dn[:], in1=pd,
                            op=mybir.AluOpType.add)
                        sb, sd = bn, dn
                    for (a, b) in scan_spans:
                        se = nc.vector
                        se.tensor_tensor_scan(
                            comp[:, a:b], sb[:, a:b],
                            (ab[:, Lpad + a:Lpad + b] if sd is None
                             else sd[:, a:b]), 0.0,
                            op0=mybir.AluOpType.mult,
                            op1=mybir.AluOpType.add)
                    if use_apg:
                        # all slot finals in ONE gpsimd indexed gather of
                        # d=2 pairs (final is the 2nd of each pair), then a
                        # strided DVE copy extracts + converts to f32
                        gath = big.tile([128, 32], sdt)
                        nc.gpsimd.ap_gather(
                            out_ap=gath[:, 0:32], in_ap=comp[:, 0:Lc],
                            idxs_ap=gidx_s[:], channels=128,
                            num_elems=Lc // 2, d=2, num_idxs=16)
                        gv = bass.AP(tensor=gath.tensor,
                                     offset=gath.offset + 1,
                                     ap=[gath.ap[0], [2, NR]])
                        nc.vector.tensor_copy(res[:], gv)
                    else:
                        for (o, j0, k, Wj) in cohorts:
                            oc, Wc = o // CF, Wj // CF
                            cv = comp[:, oc + Wc - 1:oc + Wc]
                            gap = bass.AP(tensor=cv.tensor, offset=cv.offset,
                                          ap=[cv.ap[0], [Wc, k]])
                            if gather_eng == "act":
                                nc.scalar.copy(out=res[:, j0:j0 + k],
                                               in_=gap)
                            else:
                                eng(gather_eng).tensor_copy(
                                    res[:, j0:j0 + k], gap)
                    nc.sync.dma_start(
                        out=out[:].rearrange("(k c) -> k c", c=NR),
                        in_=res[:])
    nc.finalize()
    return nc


# ---------------------------------------------------------------- entry

def _plan_key(plan):
    envs = tuple(os.environ.get(k, "") for k in (
        "GS_SCAN_DT", "GS_BETA_ENG", "GS_MUL_ENG", "GS_GATHER_ENG",
        "GS_SCAN_SPLIT", "GS_NCB", "GS_NSCAN", "GS_SKIP", "GS_ACT",
        "GS_COMPRESS", "GS_PREP_ENG", "GS_STAGGER", "GS_ONLY", "GS_MM_DT",
        "GS_MUL_GPS", "GS_BUFS", "GS_MUL_POOL", "GS_BETA_POOL",
        "GS_CULL"))
    return (plan["Lpad"], plan["NR"], plan["cohorts"], plan["groups"],
            plan["chunk_segs"], plan["ncb"], plan["scan_spans"], envs)


def _prepare(inputs, reps=1, loop_n=1):
    reps = max(reps, int(os.environ.get("GS_REPS", "1")))
    # unroll: run U renders per hardware-loop iteration so tile-pool double
    # buffering pipelines consecutive renders (loop_n is divided to keep the
    # total render count)
    U = int(os.environ.get("GS_UNROLL", "8"))
    if loop_n > 1 and U > 1:
        loop_n = max(1, loop_n // U)
        reps = reps * U
    G, colv, op, u, v = _preprocess(**inputs)
    plan, cores = _build_schedule(G, colv, op, u, v)
    key = (_plan_key(plan), reps, loop_n)
    if key not in _cache:
        _cache[key] = _build_module(key, plan, reps=reps, loop_n=loop_n)
    nc = _cache[key]
    in_maps = [{k: cores[cid][k] for k in ("ft", "g1", "g2", "colr")}
               for cid in range(NCORES)]
    if (os.environ.get("GS_GATHER_ENG", "gps") == "apg"
            and plan["CF"] == 1 and plan["NR"] <= 16):
        idx = np.zeros(16, np.int16)
        for j, (o, Wj, g, r) in enumerate(plan["slots"]):
            idx[j] = (o + Wj) // 2 - 1
        gidx = idx[np.arange(128) % 16].reshape(128, 1)
        for m in in_maps:
            m["gidx"] = gidx
    return nc, in_maps, plan


def _assemble(results, plan):
    img = np.zeros((H, W), np.float32)
    blk_of = plan["blk_of"]
    NR = plan["NR"]
    for cid in range(NCORES):
        res = results[cid]["out"].reshape(128, NR)
        for j in range(NR):
            blk = int(blk_of[cid, j])
            if blk < 0:
                continue
            by, bx = divmod(blk, NBX)
            img[by * BR:(by + 1) * BR, bx * BC:(bx + 1) * BC] = (
                res[:, j].reshape(BR, BC))
    return img.reshape(1, 1, H, W)


def kernel(**inputs):
    from concourse.bass_utils import run_bass_kernel_spmd

    inputs = {k: np.asarray(v) for k, v in inputs.items()}
    nc, in_maps, plan = _prepare(inputs)
    res = run_bass_kernel_spmd(nc, in_maps, core_ids=list(range(NCORES)))
    return _assemble(res.results, plan)



# revision 32
# speedup vs baseline: 1.2276x; 1.1862x over previous
"""Trainium2 Bass kernel for 2D Gaussian Splatting (N=1024, 256x256, 8 cores).

Math: sigma[p,i] is quadratic in pixel coords, so m1 = log(op) - sigma is
a matmul ft[6m,128pix]^T @ g[6m,cols] per 128-pixel block. Consecutive
blocks are merged into block-diagonal groups (contraction 6m <= 126; PE
cost depends only on streamed columns), and coordinates are recentered per
block so every term stays small and fp16 matmul inputs lose no accuracy
(fp32 PE matmul is ~2x slower). alpha = exp(m1) on ACT, bt = alpha*color
on DVE, beta = 1-alpha on ACT (Identity, scale=-1, bias=1); front-to-back
compositing is evaluated back-to-front as the affine scan
C = beta*C + bt along the gaussian axis.

The scan runs ~3.4 cyc/elem on DVE with no 16-bit speedup, so columns are
pre-combined 3 levels (pairs -> quads -> octs) with cheap f16 2x
tensor_tensor ops and the scan covers only L/8 columns. The column layout
is permuted so that pair members at every level are contiguous halves:
position p of a slot goes to section bitreverse3(p % 8).

Culling: 512 blocks of 8x16 pixels; a (gaussian, block) pair is kept iff
the exact minimal sigma over the block rectangle is < 5 (~3.3e-3 image
rel err vs the 2e-2 budget). Non-empty blocks are snake-dealt by width
rank onto the 8 cores (SPMD: identical program, per-core data). Slot
widths are quantized to multiples of 8 and grouped into uniform-width
cohorts; leading sentinel columns (m1=0 -> beta=0) reset the scan state
at every slot start, and one strided copy per cohort gathers the slot
finals.

The For_i timing loop uses staggered_reset (no all-engine barrier per
iteration) and unrolls 8 renders per iteration so tile-pool double
buffering pipelines consecutive renders. Inputs ship compact ([6,*] fp16
coefficient tensors) and are scattered into the block-diagonal SBUF
layout by one-time DMAs. The host reassembles the image from per-core
slot outputs; fully-culled blocks render as zero.
"""

import os
import math
import numpy as np

H = 256
W = 256
N = 1024
NCORES = 8
BR, BC = 8, 16                 # block = 8 rows x 16 cols = 128 pixels
NBY, NBX = H // BR, W // BC
NBLK = NBY * NBX               # 512
SENT_NEG = -80.0
EPS2D = 0.3
ROWS = 7                       # basis rows: x2 xy y2 x y 1 1 (F split hi/lo)
MAXROWS = 126                  # max contraction rows per merged matmul

_cache = {}


# ---------------------------------------------------------------- host math

def _preprocess(means, quats, scales, rgbs, opacities, viewmat, K):
    """Float64 per-gaussian preprocessing. Returns (in back-to-front order):
    G6 [6,N] basis coefficients of log(op)-sigma and colors [N]."""
    md = means.astype(np.float64)
    Rv = viewmat[:3, :3].astype(np.float64)
    t = viewmat[:3, 3].astype(np.float64)
    p_cam = md @ Rv.T + t
    x, y, z = p_cam[:, 0], p_cam[:, 1], p_cam[:, 2]
    fx, fy = float(K[0, 0]), float(K[1, 1])
    cx, cy = float(K[0, 2]), float(K[1, 2])
    inv_z = 1.0 / z
    u = fx * x * inv_z + cx
    v = fy * y * inv_z + cy

    th = quats.astype(np.float64)
    ct, st = np.cos(th), np.sin(th)
    zr, on = np.zeros_like(ct), np.ones_like(ct)
    R3 = np.stack([np.stack([ct, -st, zr], -1),
                   np.stack([st, ct, zr], -1),
                   np.stack([zr, zr, on], -1)], -2)
    M = R3 * scales.astype(np.float64)[:, None, :]
    cov3 = M @ np.swapaxes(M, -1, -2)
    cov_cam = np.einsum('ij,njk,lk->nil', Rv, cov3, Rv)
    j0 = np.stack([fx * inv_z, zr, -fx * x * inv_z * inv_z], -1)
    j1 = np.stack([zr, fy * inv_z, -fy * y * inv_z * inv_z], -1)
    J = np.stack([j0, j1], -2)
    cov2 = np.einsum('nij,njk,nlk->nil', J, cov_cam, J)
    a = cov2[:, 0, 0] + EPS2D
    b = cov2[:, 0, 1]
    c = cov2[:, 1, 1] + EPS2D
    det = a * c - b * b
    ca, cb, cc = c / det, -b / det, a / det

    op = 1.0 / (1.0 + np.exp(-opacities.astype(np.float64)))
    colv = 1.0 / (1.0 + np.exp(-rgbs.astype(np.float64)[:, 0]))

    # reference sorts by fp32 camera z ascending (stable); we composite
    # back-to-front = exact reverse
    order = np.argsort(z.astype(np.float32), kind="stable")
    rev = order[::-1]

    ca2, cc2 = 0.5 * ca, 0.5 * cc
    lop = np.log(op)
    d = -(ca * u + cb * v)
    e = -(cb * u + cc * v)
    f = ca2 * u * u + cb * u * v + cc2 * v * v
    G = np.stack([-ca2, -cb, -cc2, -d, -e, lop - f], 0)[:, rev]  # [6,N] f64
    return G, colv[rev], op[rev], u[rev], v[rev]


def _block_sigma_min(G, u, v):
    """Exact minimal sigma over each block rectangle: 0 if the center is
    inside, else the min over the four edges (1D quadratic, clamped)."""
    ca = -2.0 * G[0]
    cb = -G[1]
    cc = -2.0 * G[2]

    def sigma_at(dx, dy):
        return 0.5 * ca * dx * dx + cb * dx * dy + 0.5 * cc * dy * dy

    smin_all = np.zeros((NBLK, G.shape[1]))
    for by in range(NBY):
        y0, y1 = by * BR + 0.5, by * BR + BR - 0.5
        for bx in range(NBX):
            x0, x1 = bx * BC + 0.5, bx * BC + BC - 0.5
            smin = np.full(G.shape[1], np.inf)
            for xe in (x0, x1):
                dxe = xe - u
                dye = np.clip(-cb * dxe / cc, y0 - v, y1 - v)
                smin = np.minimum(smin, sigma_at(dxe, dye))
            for ye in (y0, y1):
                dye = ye - v
                dxe = np.clip(-cb * dye / ca, x0 - u, x1 - u)
                smin = np.minimum(smin, sigma_at(dxe, dye))
            inside = (u >= x0) & (u <= x1) & (v >= y0) & (v <= y1)
            smin[inside] = 0.0
            smin_all[by * NBX + bx] = smin
    return smin_all


def _rect_sigma_minmax(G, u, v, x0, x1, y0, y1):
    """Exact min and max sigma over a pixel rectangle: min via clamped 1D
    quadratics on the edges (0 if center inside), max over the corners."""
    ca = -2.0 * G[0]
    cb = -G[1]
    cc = -2.0 * G[2]

    def s_at(dx, dy):
        return 0.5 * ca * dx * dx + cb * dx * dy + 0.5 * cc * dy * dy

    smin = np.full(G.shape[1], np.inf)
    smax = np.zeros(G.shape[1])
    for xe in (x0, x1):
        dxe = xe - u
        dye = np.clip(-cb * dxe / cc, y0 - v, y1 - v)
        smin = np.minimum(smin, s_at(dxe, dye))
    for ye in (y0, y1):
        dye = ye - v
        dxe = np.clip(-cb * dye / ca, x0 - u, x1 - u)
        smin = np.minimum(smin, s_at(dxe, dye))
    inside = (u >= x0) & (u <= x1) & (v >= y0) & (v <= y1)
    smin[inside] = 0.0
    for xe in (x0, x1):
        for ye in (y0, y1):
            smax = np.maximum(smax, s_at(xe - u, ye - v))
    return smin, smax


def _block_center(blk):
    by, bx = divmod(blk, NBX)
    return bx * BC + BC / 2.0, by * BR + BR / 2.0


def _pixel_basis(blk):
    """Pixel basis recentered on the block center so all basis terms stay
    small (|dx|,|dy| <= 8) and fp16 matmul inputs lose no accuracy."""
    by, bx = divmod(blk, NBX)
    cxb, cyb = _block_center(blk)
    px = np.arange(W, dtype=np.float64) + 0.5 - cxb
    py = np.arange(H, dtype=np.float64) + 0.5 - cyb
    gy, gx = np.meshgrid(py[by * BR:(by + 1) * BR],
                         px[bx * BC:(bx + 1) * BC], indexing="ij")
    fxr, fyr = gx.ravel(), gy.ravel()
    on = np.ones_like(fxr)
    return np.stack([fxr * fxr, fxr * fyr, fyr * fyr, fxr, fyr,
                     on, on], 0).astype(np.float32)   # [ROWS,128]


def _recentered_coeffs(G, idx, blk, extra_const=None):
    """Per-(block, gaussian) polynomial coefficients of m1 in block-centered
    coordinates: m1 = A dx^2 + B dxdy + C dy^2 + D dx + E dy + F."""
    cxb, cyb = _block_center(blk)
    A, B, C = G[0][idx], G[1][idx], G[2][idx]
    d_, e_, f_ = G[3][idx], G[4][idx], G[5][idx]
    if extra_const is not None:
        f_ = f_ + extra_const
    D = 2 * A * cxb + B * cyb + d_
    E = B * cxb + 2 * C * cyb + e_
    F = (A * cxb * cxb + B * cxb * cyb + C * cyb * cyb
         + d_ * cxb + e_ * cyb + f_)
    F = np.maximum(F, SENT_NEG)
    # split the constant term so the f16 matmul keeps full precision on F
    # (|F| up to 80 has f16 ulp 0.06; the PSUM accumulates hi+lo in f32)
    F_hi = F.astype(np.float16).astype(np.float64)
    F_lo = F - F_hi
    return np.stack([A, B, C, D, E, F_hi, F_lo], 0).astype(np.float32)


def _build_schedule(G, colv, op, u, v):
    T = float(os.environ.get("GS_T", "4.0"))
    QW = int(os.environ.get("GS_QW", "8"))
    smin = _block_sigma_min(G, u, v)
    cull = os.environ.get("GS_CULL", "plain")
    if cull == "w":
        # weighted: cull when op*c*e^-smin < op_max*c_max*e^-T, i.e. dimmer
        # gaussians are culled at smaller sigma
        opc = op * colv
        thr = T - np.log(opc.max() / opc)
        masks = smin < thr[None, :]
    elif cull == "o":
        # occlusion-aware at sub-block resolution: keep (front-to-back)
        # iff for SOME sub-rectangle r the contribution bound
        # op*c*e^-smin_r * T_ub_r >= opc_ref*e^-T holds, where T_ub_r is
        # the product of (1 - op*e^-smax_r) over kept closer gaussians
        OS = int(os.environ.get("GS_OSUB", "2"))
        NG = G.shape[1]
        RR = OS * OS
        smin_sub = np.zeros((NBLK, RR, NG))
        smax_sub = np.zeros((NBLK, RR, NG))
        for by in range(NBY):
            for bx in range(NBX):
                b = by * NBX + bx
                r = 0
                for sy in range(OS):
                    y0 = by * BR + sy * BR // OS + 0.5
                    y1 = by * BR + (sy + 1) * BR // OS - 0.5
                    for sx in range(OS):
                        x0 = bx * BC + sx * BC // OS + 0.5
                        x1 = bx * BC + (sx + 1) * BC // OS - 0.5
                        smin_sub[b, r], smax_sub[b, r] = _rect_sigma_minmax(
                            G, u, v, x0, x1, y0, y1)
                        r += 1
        opc = op * colv
        lref = np.log(opc.max())
        lw = np.log(opc)
        masks = np.zeros_like(smin, bool)
        with np.errstate(divide="ignore"):
            l1a = np.log1p(-np.minimum(op * np.exp(-smax_sub), 0.999))
        for b in range(NBLK):
            keep = masks[b]
            sb = smin_sub[b]            # [RR, NG]
            xb = l1a[b]
            cand = np.nonzero(lw - sb.min(0) >= lref - T)[0]
            lT = np.zeros(RR)
            for i in cand[::-1]:
                if np.any(lw[i] - sb[:, i] + lT >= lref - T):
                    keep[i] = True
                    lT += xb[:, i]
                    if lT.max() < -T:
                        break       # saturated: nothing behind can pass
    else:
        masks = smin < T
    widths = masks.sum(1)

    nz = np.nonzero(widths)[0]
    order = nz[np.argsort(widths[nz], kind="stable")[::-1]]
    nnz = len(order)
    NR = (nnz + NCORES - 1) // NCORES          # ranks (slots per core)
    blk_of = np.full((NCORES, NR), -1, np.int64)
    for j in range(NR):
        grp = order[j * NCORES:(j + 1) * NCORES]
        if j % 2 == 1:
            grp = grp[::-1]
        # place into cores (tail rank may be short)
        if j % 2 == 1 and len(grp) < NCORES:
            blk_of[NCORES - len(grp):, j] = grp
        else:
            blk_of[:len(grp), j] = grp
    rank_w = np.zeros(NR, np.int64)
    for j in range(NR):
        bs = blk_of[:, j]
        rank_w[j] = max(widths[b] if b >= 0 else 0 for b in bs)
    slot_W = ((rank_w + 1 + QW - 1) // QW) * QW     # >=1 leading sentinel

    # cohorts: consecutive ranks sharing the same quantized width
    cohorts = []                                    # (off, j0, k, W)
    off = 0
    j = 0
    while j < NR:
        j0 = j
        Wj = int(slot_W[j])
        while j < NR and slot_W[j] == Wj:
            j += 1
        cohorts.append((off, j0, j - j0, Wj))
        off += (j - j0) * Wj
    L = off
    Lpad = (L + 511) // 512 * 512
    S = Lpad // 512

    slot_off = np.zeros(NR, np.int64)
    for (o, j0, k, Wj) in cohorts:
        for r in range(k):
            slot_off[j0 + r] = o + r * Wj

    # compression: level v scans L/2^v columns after v rounds of pair
    # combination. The column layout is permuted so pair members at every
    # level are contiguous halves: position p of a slot goes to section
    # bitreverse_v(p % 2^v) (e.g. v=2 -> [0,2,1,3], v=3 -> [0,4,2,6,1,5,3,7]).
    comp_lv = int(os.environ.get("GS_COMPRESS", "3"))
    CF = 1 << comp_lv                       # compression factor
    sec_of = np.array([int(format(r, f"0{max(comp_lv,1)}b")[::-1], 2)
                       for r in range(CF)])

    # block-diagonal matmul groups over consecutive ranks
    groups = []                                     # (j0, m)
    j = 0
    while j < NR:
        m = min(MAXROWS // ROWS, NR - j)
        groups.append((j, m))
        j += m
    grp_of = np.zeros(NR, np.int64)
    rank_in_grp = np.zeros(NR, np.int64)
    for g, (j0, m) in enumerate(groups):
        for r in range(m):
            grp_of[j0 + r] = g
            rank_in_grp[j0 + r] = r

    # segments: group-section spans intersected with the 512 grid; the tail
    # pad [L, Lpad) is appended to the last section (zero rhs -> m=0).
    # With compression the layout has CF sections of L/CF columns each and
    # slot j's section-k subrange is [k*L/CF + o/CF, k*L/CF + (o+W)/CF).
    LpC = L // CF
    spans = []
    for g, (j0, m) in enumerate(groups):
        for k in range(CF):
            a = k * LpC + int(slot_off[j0]) // CF
            b = k * LpC + int(slot_off[j0 + m - 1] + slot_W[j0 + m - 1]) // CF
            if g == len(groups) - 1 and k == CF - 1:
                b = Lpad
            spans.append((g, a, b))
    # merge adjacent spans with the same group (sections of one group tile
    # the layout contiguously), then split on the 512 PSUM-chunk grid
    spans.sort(key=lambda s: s[1])
    merged = []
    for g, a, b in spans:
        if merged and merged[-1][0] == g and merged[-1][2] == a:
            merged[-1][2] = b
        else:
            merged.append([g, a, b])
    seg_list = []
    for g, a, b in merged:
        while a < b:
            nb = min(b, (a // 512 + 1) * 512)
            seg_list.append((g, a, nb))
            a = nb
    chunk_segs = [[] for _ in range(S)]
    for g, a, b in seg_list:
        chunk_segs[a // 512].append((g, a, b))

    lcol = np.log(colv)
    mm_f16 = os.environ.get("GS_MM_DT", "f16") == "f16"

    # chunk kinds: first NCB chunks compute bt = alpha*color on a vector
    # engine (cb); the rest use a second matmul + exp (mm2). -1 = all cb.
    ncb = int(os.environ.get("GS_NCB", "-1"))
    ncb = S if ncb < 0 else min(ncb, S)

    # scan spans: merge cohorts into ~GS_NSCAN contiguous scans (in the
    # compressed domain [0, L/CF); slot j occupies [o/CF, (o+W)/CF))
    nscan = int(os.environ.get("GS_NSCAN", "12"))
    Lc = L // CF
    tgt = max(1, (Lc + nscan - 1) // max(1, nscan))
    scan_spans = []
    cur_a = 0
    for (o, j0, k, Wj) in cohorts:
        end = (o + k * Wj) // CF
        if end - cur_a >= tgt or (o, j0, k, Wj) == cohorts[-1]:
            scan_spans.append((cur_a, end))
            cur_a = end
    if cur_a < Lc:
        scan_spans.append((cur_a, Lc))

    ngroups = len(groups)
    cores = []
    for cid in range(NCORES):
        g1 = np.zeros((ROWS, Lpad), np.float32)
        g2 = np.zeros((ROWS, Lpad), np.float32)
        colr = np.zeros(Lpad, np.float32)
        ftc = np.zeros((ROWS, NR * 128), np.float32)
        for j in range(NR):
            o = int(slot_off[j])
            Wj = int(slot_W[j])
            blk = int(blk_of[cid, j])
            p = np.arange(Wj)
            dest = sec_of[p % CF] * (L // CF) + o // CF + p // CF
            if blk >= 0:
                idx = np.nonzero(masks[blk])[0]
                nb = len(idx)
                g1[:, dest[Wj - nb:]] = _recentered_coeffs(G, idx, blk)
                g2[:, dest[Wj - nb:]] = _recentered_coeffs(
                    G, idx, blk, extra_const=lcol[idx])
                colr[dest[Wj - nb:]] = colv[idx].astype(np.float32)
                ftc[:, j * 128:(j + 1) * 128] = _pixel_basis(blk)
            else:
                nb = 0
                ftc[5, j * 128:(j + 1) * 128] = 1.0
            # leading sentinels: m1 = 0 (alpha=1 -> beta=0), m2 = -80 (bt~0)
            g2[5, dest[:Wj - nb]] = SENT_NEG
        if mm_f16:
            g1 = g1.astype(np.float16)
            g2 = g2.astype(np.float16)
            ftc = ftc.astype(np.float16)
        if os.environ.get("GS_SCAN_DT", "f16") == "f16":
            colr = colr.astype(np.float16)
        cores.append({"ft": ftc, "g1": g1, "g2": g2, "colr": colr})

    plan = {
        "Lpad": Lpad, "S": S, "NR": NR, "ngroups": ngroups, "L": L,
        "ncb": ncb, "CF": CF,
        "scan_spans": tuple(scan_spans),
        "cohorts": tuple(cohorts),
        "groups": tuple(groups),
        "chunk_segs": tuple(tuple(c) for c in chunk_segs),
        "slots": tuple((int(slot_off[j]), int(slot_W[j]), int(grp_of[j]),
                        int(rank_in_grp[j])) for j in range(NR)),
        "blk_of": blk_of,
    }
    return plan, cores


# ---------------------------------------------------------------- device

def _build_module(key, plan, reps=1, loop_n=1):
    import contextlib
    import concourse.bass as bass
    import concourse.bacc as bacc
    import concourse.tile as tile
    from concourse import mybir

    f32 = mybir.dt.float32
    sdt = {"f32": mybir.dt.float32, "f16": mybir.dt.float16,
           "bf16": mybir.dt.bfloat16}[os.environ.get("GS_SCAN_DT", "f16")]
    beta_eng = os.environ.get("GS_BETA_ENG", "act")
    mul_eng = os.environ.get("GS_MUL_ENG", "dve")
    gather_eng = os.environ.get("GS_GATHER_ENG", "gps")

    Lpad, S, NR = plan["Lpad"], plan["S"], plan["NR"]
    ngroups, ncb = plan["ngroups"], plan["ncb"]
    cohorts, groups = plan["cohorts"], plan["groups"]
    chunk_segs = plan["chunk_segs"]
    scan_spans = plan["scan_spans"]
    slots = plan["slots"]
    hmax = ROWS * max(m for _, m in groups)
    skip = set(os.environ.get("GS_SKIP", "").split(","))

    mdt = (mybir.dt.float16 if os.environ.get("GS_MM_DT", "f16") == "f16"
           else f32)

    use_apg = (gather_eng == "apg" and plan["CF"] == 1 and NR <= 16)
    nc = bacc.Bacc(None)
    ft = nc.dram_tensor("ft", [ROWS, NR * 128], mdt, kind="ExternalInput")
    if use_apg:
        gidx = nc.dram_tensor("gidx", [128, 1], mybir.dt.int16,
                              kind="ExternalInput")
    g1 = nc.dram_tensor("g1", [ROWS, Lpad], mdt, kind="ExternalInput")
    g2 = nc.dram_tensor("g2", [ROWS, Lpad], mdt, kind="ExternalInput")
    cdt = mybir.dt.float16 if sdt == mybir.dt.float16 else f32
    colr = nc.dram_tensor("colr", [Lpad], cdt, kind="ExternalInput")
    out = nc.dram_tensor("out", [128 * NR], f32, kind="ExternalOutput")

    def eng(name):
        return {"dve": nc.vector, "gps": nc.gpsimd, "act": nc.scalar}[name]

    with tile.TileContext(nc) as tc:
        with (
            tc.tile_pool(name="const", bufs=1) as consts,
            tc.tile_pool(name="big",
                         bufs=int(os.environ.get("GS_BUFS", "2"))) as big,
            tc.tile_pool(name="psum",
                         bufs=2 if (Lpad <= 1024 and ncb < S) else 4,
                         space="PSUM") as psum,
        ):
            # scatter compact [6,*] inputs into block-diagonal SBUF layout
            ft_s = consts.tile([hmax, ngroups * 128], mdt)
            r1_s = consts.tile([hmax, Lpad], mdt)
            r2_s = consts.tile([hmax, Lpad], mdt)
            nc.vector.memset(r1_s[:], 0.0)
            nc.vector.memset(r2_s[:], 0.0)
            CF, L = plan["CF"], plan["L"]
            for j, (o, Wj, g, r) in enumerate(slots):
                nc.sync.dma_start(
                    out=ft_s[ROWS * r:ROWS * (r + 1), g * 128:(g + 1) * 128],
                    in_=ft[:, j * 128:(j + 1) * 128])
                for k in range(CF):
                    a = k * (L // CF) + o // CF
                    b = a + Wj // CF
                    nc.sync.dma_start(out=r1_s[ROWS * r:ROWS * (r + 1), a:b],
                                      in_=g1[:, a:b])
                    nc.sync.dma_start(out=r2_s[ROWS * r:ROWS * (r + 1), a:b],
                                      in_=g2[:, a:b])
            if ncb > 0:
                c_s = consts.tile([128, ncb * 512], cdt)
                cseg = colr[0:ncb * 512]
                bc = bass.AP(tensor=cseg.tensor, offset=cseg.offset,
                             ap=[[0, 128], cseg.ap[0]])
                nc.sync.dma_start(out=c_s[:], in_=bc)
            if use_apg:
                gidx_s = consts.tile([128, 1], mybir.dt.int16)
                nc.sync.dma_start(out=gidx_s[:], in_=gidx[:])

            only = os.environ.get("GS_ONLY", "")
            Lc0 = L // plan["CF"]
            if only == "scan":
                sbc = consts.tile([128, Lc0], sdt)
                sdc = consts.tile([128, Lc0], sdt)
                nc.vector.memset(sbc[:], 0.5)
                nc.vector.memset(sdc[:], 0.01)
            elif only == "beta":
                alc = consts.tile([128, Lpad], sdt)
                nc.vector.memset(alc[:], 0.5)

            loop_cm = (
                tc.For_i(0, loop_n, 1, hint_engines=(
                    mybir.EngineType.PE, mybir.EngineType.Activation,
                    mybir.EngineType.DVE, mybir.EngineType.Pool),
                    staggered_reset=os.environ.get("GS_STAGGER", "1") == "1")
                if loop_n > 1 else contextlib.nullcontext()
            )
            Lc = L // CF
            prep_eng = os.environ.get("GS_PREP_ENG", "dve")
            with loop_cm:
                for _ in range(reps):
                    if only:
                        res = big.tile([128, NR], f32)
                        if only == "scan":
                            comp = big.tile([128, Lc], sdt)
                            for (a, b) in scan_spans:
                                nc.vector.tensor_tensor_scan(
                                    comp[:, a:b], sbc[:, a:b], sdc[:, a:b],
                                    0.0, op0=mybir.AluOpType.mult,
                                    op1=mybir.AluOpType.add)
                            for (o, j0, k, Wj) in cohorts:
                                oc, Wc = o // CF, Wj // CF
                                cv = comp[:, oc + Wc - 1:oc + Wc]
                                gap = bass.AP(
                                    tensor=cv.tensor, offset=cv.offset,
                                    ap=[cv.ap[0], [Wc, k]])
                                nc.vector.tensor_copy(res[:, j0:j0 + k], gap)
                        elif only == "mm":
                            for s in range(S):
                                ps = psum.tile([128, 1024], f32)
                                for g, a, b in chunk_segs[s]:
                                    h = ROWS * groups[g][1]
                                    lhs = ft_s[0:h, g * 128:(g + 1) * 128]
                                    nc.tensor.matmul(
                                        ps[:, a - s * 512:b - s * 512],
                                        lhsT=lhs, rhs=r1_s[0:h, a:b],
                                        start=True, stop=True)
                                    nc.tensor.matmul(
                                        ps[:, 512 + a - s * 512:
                                           512 + b - s * 512],
                                        lhsT=lhs, rhs=r2_s[0:h, a:b],
                                        start=True, stop=True)
                            nc.scalar.copy(out=res[:], in_=ps[:, 0:NR])
                        elif only == "mmwide":
                            # timing probe: one 1024-col matmul per chunk
                            # (rhs content nonsense, timing valid)
                            h = ROWS * groups[0][1]
                            lhs = ft_s[0:h, 0:128]
                            for s in range(S):
                                ps = psum.tile([128, 1024], f32)
                                w = min(1024, Lpad - s * 512)
                                nc.tensor.matmul(
                                    ps[:, 0:w], lhsT=lhs,
                                    rhs=r1_s[0:h, s * 512:s * 512 + w],
                                    start=True, stop=True)
                            nc.scalar.copy(out=res[:], in_=ps[:, 0:NR])
                        elif only == "exp":
                            ab = big.tile([128, 2 * Lpad], sdt)
                            for s in range(S):
                                ps = psum.tile([128, 1024], f32)
                                for g, a, b in chunk_segs[s]:
                                    h = ROWS * groups[g][1]
                                    lhs = ft_s[0:h, g * 128:(g + 1) * 128]
                                    nc.tensor.matmul(
                                        ps[:, a - s * 512:b - s * 512],
                                        lhsT=lhs, rhs=r1_s[0:h, a:b],
                                        start=True, stop=True)
                                    nc.tensor.matmul(
                                        ps[:, 512 + a - s * 512:
                                           512 + b - s * 512],
                                        lhsT=lhs, rhs=r2_s[0:h, a:b],
                                        start=True, stop=True)
                                nc.scalar.activation(
                                    out=ab[:, s * 512:(s + 1) * 512],
                                    in_=ps[:, 0:512],
                                    func=mybir.ActivationFunctionType.Exp,
                                    scale=1.0, bias=0.0)
                                nc.scalar.activation(
                                    out=ab[:, Lpad + s * 512:
                                           Lpad + (s + 1) * 512],
                                    in_=ps[:, 512:1024],
                                    func=mybir.ActivationFunctionType.Exp,
                                    scale=1.0, bias=0.0)
                            nc.vector.tensor_copy(res[:], ab[:, 0:NR])
                        elif only == "beta":
                            beta = big.tile([128, Lpad], sdt)
                            for s in range(S):
                                eng(beta_eng).tensor_scalar(
                                    out=beta[:, s * 512:(s + 1) * 512],
                                    in0=alc[:, s * 512:(s + 1) * 512],
                                    scalar1=-1.0, scalar2=1.0,
                                    op0=mybir.AluOpType.mult,
                                    op1=mybir.AluOpType.add)
                            nc.vector.tensor_copy(res[:], beta[:, 0:NR])
                        nc.sync.dma_start(
                            out=out[:].rearrange("(k c) -> k c", c=NR),
                            in_=res[:])
                        continue
                    ab = big.tile([128, 2 * Lpad], sdt)   # alpha | bt
                    beta = big.tile([128, Lpad], sdt)
                    comp = big.tile([128, Lc], sdt)
                    res = big.tile([128, NR], f32)
                    # chunk pairs: fused PSUM tiles, one exp per pair, all
                    # per-chunk ops trimmed at L (the tail [L, Lpad) is never
                    # read by compress/scan)
                    if Lpad <= 1024:
                        # whole render in ONE PSUM tile: m1 of all chunks
                        # at [0, Lpad), m2 of mm2 chunks at [Lpad, 2Lpad)
                        # (all offsets 512-aligned -> no matmul crosses a
                        # bank). One fused exp covers every alpha.
                        mm2 = ncb < S
                        ps = psum.tile([128, (2 if mm2 else 1) * Lpad], f32)
                        if "mm" not in skip:
                            for s in range(S):
                                for g, a, b in chunk_segs[s]:
                                    b = min(b, L)
                                    if a >= b:
                                        continue
                                    h = ROWS * groups[g][1]
                                    lhs = ft_s[0:h, g * 128:(g + 1) * 128]
                                    nc.tensor.matmul(
                                        ps[:, a:b], lhsT=lhs,
                                        rhs=r1_s[0:h, a:b],
                                        start=True, stop=True)
                                    if s >= ncb:
                                        nc.tensor.matmul(
                                            ps[:, Lpad + a:Lpad + b],
                                            lhsT=lhs, rhs=r2_s[0:h, a:b],
                                            start=True, stop=True)
                        if "exp" not in skip:
                            nc.scalar.activation(
                                out=ab[:, 0:L], in_=ps[:, 0:L],
                                func=mybir.ActivationFunctionType.Exp,
                                scale=1.0, bias=0.0)
                            for s in range(ncb, S):
                                a0 = s * 512
                                ew = min(512, L - a0)
                                nc.scalar.activation(
                                    out=ab[:, Lpad + a0:Lpad + a0 + ew],
                                    in_=ps[:, Lpad + a0:Lpad + a0 + ew],
                                    func=mybir.ActivationFunctionType.Exp,
                                    scale=1.0, bias=0.0)
                    else:
                        p = 0
                        while p < S:
                            cb = p < ncb
                            # cb chunks pair up ([m1|m1] in one 2-bank
                            # tile); mm2 chunks stay single ([m1|m2], also
                            # 2 banks) so every tile is [128, 1024]
                            np_ = min(2, ncb - p) if cb else 1
                            a0 = p * 512
                            ew = min(np_ * 512, L - a0)
                            ps = psum.tile([128, 1024], f32)
                            if "mm" not in skip:
                                for s in range(p, p + np_):
                                    for g, a, b in chunk_segs[s]:
                                        b = min(b, L)
                                        if a >= b:
                                            continue
                                        h = ROWS * groups[g][1]
                                        lhs = ft_s[0:h,
                                                   g * 128:(g + 1) * 128]
                                        nc.tensor.matmul(
                                            ps[:, a - a0:b - a0],
                                            lhsT=lhs, rhs=r1_s[0:h, a:b],
                                            start=True, stop=True)
                                        if not cb:
                                            # m2 at bank-aligned offset 512
                                            nc.tensor.matmul(
                                                ps[:, 512 + a - a0:
                                                   512 + b - a0],
                                                lhsT=lhs,
                                                rhs=r2_s[0:h, a:b],
                                                start=True, stop=True)
                            av = ab[:, a0:a0 + ew]
                            if "exp" in skip:
                                pass
                            elif cb:
                                nc.scalar.activation(
                                    out=av, in_=ps[:, 0:ew],
                                    func=mybir.ActivationFunctionType.Exp,
                                    scale=1.0, bias=0.0)
                            else:
                                nc.scalar.activation(
                                    out=av, in_=ps[:, 0:ew],
                                    func=mybir.ActivationFunctionType.Exp,
                                    scale=1.0, bias=0.0)
                                nc.scalar.activation(
                                    out=ab[:, Lpad + a0:Lpad + a0 + ew],
                                    in_=ps[:, 512:512 + ew],
                                    func=mybir.ActivationFunctionType.Exp,
                                    scale=1.0, bias=0.0)
                            p += np_
                    ncb_cols = min(ncb * 512, L)
                    if ncb_cols > 0 and "exp" not in skip:
                        # bt = alpha * color, optionally split DVE / Pool
                        mk = int(float(os.environ.get("GS_MUL_POOL", "0"))
                                 * ncb_cols)
                        if mk > 0:
                            nc.gpsimd.tensor_tensor(
                                out=ab[:, Lpad:Lpad + mk],
                                in0=ab[:, 0:mk], in1=c_s[:, 0:mk],
                                op=mybir.AluOpType.mult)
                        if mk < ncb_cols:
                            eng(mul_eng).tensor_tensor(
                                out=ab[:, Lpad + mk:Lpad + ncb_cols],
                                in0=ab[:, mk:ncb_cols],
                                in1=c_s[:, mk:ncb_cols],
                                op=mybir.AluOpType.mult)
                    if "beta" not in skip:
                        # beta = 1 - alpha, per chunk so scan spans can
                        # start as soon as their range is ready
                        for s in range(S):
                            b0, b1 = s * 512, min((s + 1) * 512, L)
                            if b0 >= b1:
                                continue
                            if beta_eng == "act":
                                nc.scalar.activation(
                                    out=beta[:, b0:b1], in_=ab[:, b0:b1],
                                    func=(mybir.ActivationFunctionType
                                          .Identity),
                                    scale=-1.0, bias=1.0)
                            else:
                                eng(beta_eng).tensor_scalar(
                                    out=beta[:, b0:b1], in0=ab[:, b0:b1],
                                    scalar1=-1.0, scalar2=1.0,
                                    op0=mybir.AluOpType.mult,
                                    op1=mybir.AluOpType.add)
                    sb, sd = beta, None     # sd None -> bt lives in ab
                    half = L
                    pe = eng(prep_eng)
                    while half > Lc:
                        half //= 2
                        bn = big.tile([128, half], sdt)
                        dn = big.tile([128, half], sdt)
                        pb = sb[:, half:2 * half]
                        pd = (ab[:, Lpad + half:Lpad + 2 * half]
                              if sd is None else sd[:, half:2 * half])
                        pe.tensor_tensor(
                            out=bn[:], in0=sb[:, 0:half], in1=pb,
                            op=mybir.AluOpType.mult)
                        pe.tensor_tensor(
                            out=dn[:],
                            in0=(ab[:, Lpad:Lpad + half]
                                 if sd is None else sd[:, 0:half]),
                            in1=pb, op=mybir.AluOpType.mult)
                        pe.tensor_tensor(
                            out=dn[:], in0=dn[:], in1=pd,
                            op=mybir.AluOpType.add)
                        sb, sd = bn, dn
                    for (a, b) in scan_spans:
                        se = nc.vector
                        se.tensor_tensor_scan(
                            comp[:, a:b], sb[:, a:b],
                            (ab[:, Lpad + a:Lpad + b] if sd is None
                             else sd[:, a:b]), 0.0,
                            op0=mybir.AluOpType.mult,
                            op1=mybir.AluOpType.add)
                    if use_apg:
                        # all slot finals in ONE gpsimd indexed gather of
                        # d=2 pairs (final is the 2nd of each pair), then a
                        # strided DVE copy extracts + converts to f32
                        gath = big.tile([128, 32], sdt)
                        nc.gpsimd.ap_gather(
                            out_ap=gath[:, 0:32], in_ap=comp[:, 0:Lc],
                            idxs_ap=gidx_s[:], channels=128,
                            num_elems=Lc // 2, d=2, num_idxs=16)
                        gv = bass.AP(tensor=gath.tensor,
                                     offset=gath.offset + 1,
                                     ap=[gath.ap[0], [2, NR]])
                        nc.vector.tensor_copy(res[:], gv)
                    else:
                        for (o, j0, k, Wj) in cohorts:
                            oc, Wc = o // CF, Wj // CF
                            cv = comp[:, oc + Wc - 1:oc + Wc]
                            gap = bass.AP(tensor=cv.tensor, offset=cv.offset,
                                          ap=[cv.ap[0], [Wc, k]])
                            if gather_eng == "act":
                                nc.scalar.copy(out=res[:, j0:j0 + k],
                                               in_=gap)
                            else:
                                eng(gather_eng).tensor_copy(
                                    res[:, j0:j0 + k], gap)
                    nc.sync.dma_start(
                        out=out[:].rearrange("(k c) -> k c", c=NR),
                        in_=res[:])
    nc.finalize()
    return nc


# ---------------------------------------------------------------- entry

def _plan_key(plan):
    envs = tuple(os.environ.get(k, "") for k in (
        "GS_SCAN_DT", "GS_BETA_ENG", "GS_MUL_ENG", "GS_GATHER_ENG",
        "GS_SCAN_SPLIT", "GS_NCB", "GS_NSCAN", "GS_SKIP", "GS_ACT",
        "GS_COMPRESS", "GS_PREP_ENG", "GS_STAGGER", "GS_ONLY", "GS_MM_DT",
        "GS_MUL_GPS", "GS_BUFS", "GS_MUL_POOL", "GS_BETA_POOL",
        "GS_CULL"))
    return (plan["Lpad"], plan["NR"], plan["cohorts"], plan["groups"],
            plan["chunk_segs"], plan["ncb"], plan["scan_spans"], envs)


def _prepare(inputs, reps=1, loop_n=1):
    reps = max(reps, int(os.environ.get("GS_REPS", "1")))
    # unroll: run U renders per hardware-loop iteration so tile-pool double
    # buffering pipelines consecutive renders (loop_n is divided to keep the
    # total render count)
    U = int(os.environ.get("GS_UNROLL", "8"))
    if loop_n > 1 and U > 1:
        loop_n = max(1, loop_n // U)
        reps = reps * U
    G, colv, op, u, v = _preprocess(**inputs)
    plan, cores = _build_schedule(G, colv, op, u, v)
    key = (_plan_key(plan), reps, loop_n)
    if key not in _cache:
        _cache[key] = _build_module(key, plan, reps=reps, loop_n=loop_n)
    nc = _cache[key]
    in_maps = [{k: cores[cid][k] for k in ("ft", "g1", "g2", "colr")}
               for cid in range(NCORES)]
    if (os.environ.get("GS_GATHER_ENG", "gps") == "apg"
            and plan["CF"] == 1 and plan["NR"] <= 16):
        idx = np.zeros(16, np.int16)
        for j, (o, Wj, g, r) in enumerate(plan["slots"]):
            idx[j] = (o + Wj) // 2 - 1
        gidx = idx[np.arange(128) % 16].reshape(128, 1)
        for m in in_maps:
            m["gidx"] = gidx
    return nc, in_maps, plan


def _assemble(results, plan):
    img = np.zeros((H, W), np.float32)
    blk_of = plan["blk_of"]
    NR = plan["NR"]
    for cid in range(NCORES):
        res = results[cid]["out"].reshape(128, NR)
        for j in range(NR):
            blk = int(blk_of[cid, j])
            if blk < 0:
                continue
            by, bx = divmod(blk, NBX)
            img[by * BR:(by + 1) * BR, bx * BC:(bx + 1) * BC] = (
                res[:, j].reshape(BR, BC))
    return img.reshape(1, 1, H, W)


def kernel(**inputs):
    from concourse.bass_utils import run_bass_kernel_spmd

    inputs = {k: np.asarray(v) for k, v in inputs.items()}
    nc, in_maps, plan = _prepare(inputs)
    res = run_bass_kernel_spmd(nc, in_maps, core_ids=list(range(NCORES)))
    return _assemble(res.results, plan)

